# revision 1
# baseline (speedup 1.0000x reference)
"""Trainium2 Bass kernel for EnhancedGraphAttentionLayer (B=1, N=1024, D=64).

Sharding: destination-node rows split across 8 cores (128 rows each).
Each core is fully independent (no collectives): it holds h replicated and
computes its 128 rows of scores/softmax/attention locally.

Algorithm per core (row block R of 128 destination nodes i):
  Decompose LeakyReLU(x) = 0.2*x + 0.8*relu(x) at both nonlinearities so the
  0.2-linear parts fold into matmuls:
    edge@A_e with edge = LRelu(ei+ej+b):
      t+v = 0.8*A_e^T relu(s) + Mv^T h_j   (Mv = W@A_j + 0.2*E_j@A_e)
      per-i bias u = q_i + b1 + 0.2*A_e^T(ei_i + b)
    e = w2^T LRelu(pre), pre = t+v+u:
      e = 0.8*w2^T relu(pre) + 0.2*w2^T pre (+ row-const, dropped: softmax-inv.)
      0.2*w2^T(t+v) computed as column 64 of the main matmul; +4096 bias keeps
      the relu pass-through on that row; the +4096 per-row constant cancels in
      softmax. Scores accumulate in PSUM via shifted one-hot weight columns.
  Mask = multiply by {0,1} adj (scores are ~4096+eps>0; masked 0 underflows in
  softmax exactly like the reference's -1e9).
"""
import sys
import numpy as np

if "/opt/trn_rl_repo" not in sys.path:
    sys.path.insert(0, "/opt/trn_rl_repo")

import ml_dtypes
import concourse.bass as bass
import concourse.bacc as bacc
import concourse.mybir as mybir
import concourse.tile as tile
from concourse.bass_utils import run_bass_kernel_spmd

F32 = mybir.dt.float32
BF16 = mybir.dt.bfloat16
AF = mybir.ActivationFunctionType
ALU = mybir.AluOpType
AX = mybir.AxisListType

N = 1024
D = 64
NCORES = 8
R = N // NCORES          # 128 rows per core
ALPHA = 0.2
C_ROW64 = 4096.0         # relu-safe shift on the linear-score row
LN_EPS = 1e-5

_CACHE = {}


def _build_program():
    nc = bacc.Bacc("TRN2", target_bir_lowering=False, debug=False,
                   num_devices=NCORES)

    def din(name, shape, dt):
        return nc.dram_tensor(name, shape, dt, kind="ExternalInput").ap()

    hT_f = din("hT_f", [D, N], F32)
    hT_bf = din("hT_bf", [D, N], BF16)
    hTr = din("hTr", [D, R], F32)
    hrows = din("hrows", [R, D], F32)
    adjf = din("adjf", [R, N], F32)
    lhsT1 = din("lhsT1", [2 * D, D + 1], BF16)
    lhsT2u = din("lhsT2u", [D + 1, 32 * 32], F32)
    Ej = din("Ej", [D, D], F32)
    Ei = din("Ei", [D, D], F32)
    Wm = din("Wm", [D, D], F32)
    Ai = din("Ai", [D, D], F32)
    Ae = din("Ae", [D, D], F32)
    b1col = din("b1col", [D, 1], F32)
    ebcol = din("ebcol", [D, 1], F32)
    iden = din("iden", [128, 128], F32)
    lngr = din("lngr", [R, D], F32)
    lnbr = din("lnbr", [R, D], F32)
    out_d = nc.dram_tensor("out", [R, D], F32, kind="ExternalOutput").ap()

    with tile.TileContext(nc) as tc, \
         tc.tile_pool(name="static", bufs=1) as sp:
        # ---------------- static SBUF tiles ----------------
        hT_sb = sp.tile([D, N], F32, name="hT_sb", tag="hT_sb")
        hTr_sb = sp.tile([D, R], F32, name="hTr_sb", tag="hTr_sb")
        hrows_sb = sp.tile([R, D], F32, name="hrows_sb", tag="hrows_sb")
        adjf_sb = sp.tile([R, N], F32, name="adjf_sb", tag="adjf_sb")
        lhsT1_sb = sp.tile([2 * D, D + 1], BF16, name="lhsT1_sb", tag="lhsT1_sb")
        lhsT2u_sb = sp.tile([D + 1, 32 * 32], F32, name="lhsT2u_sb", tag="lhsT2u_sb")
        Ej_sb = sp.tile([D, D], F32, name="Ej_sb", tag="Ej_sb")
        Ei_sb = sp.tile([D, D], F32, name="Ei_sb", tag="Ei_sb")
        Wm_sb = sp.tile([D, D], F32, name="Wm_sb", tag="Wm_sb")
        Ai_sb = sp.tile([D, D], F32, name="Ai_sb", tag="Ai_sb")
        Ae_sb = sp.tile([D, D], F32, name="Ae_sb", tag="Ae_sb")
        b1_sb = sp.tile([D, 1], F32, name="b1_sb", tag="b1_sb")
        eb_sb = sp.tile([D, 1], F32, name="eb_sb", tag="eb_sb")
        iden_sb = sp.tile([128, 128], F32, name="iden_sb", tag="iden_sb")
        lngr_sb = sp.tile([R, D], F32, name="lngr_sb", tag="lngr_sb")
        lnbr_sb = sp.tile([R, D], F32, name="lnbr_sb", tag="lnbr_sb")

        ejT_bf_sb = sp.tile([D, N], BF16, name="ejT_bf_sb", tag="ejT_bf_sb")
        eibr_sb = sp.tile([D, R], F32, name="eibr_sb", tag="eibr_sb")
        WhTr_sb = sp.tile([D, R], F32, name="WhTr_sb", tag="WhTr_sb")
        qb_sb = sp.tile([D, R], F32, name="qb_sb", tag="qb_sb")
        u_sb = sp.tile([D + 1, R], F32, name="u_sb", tag="u_sb")
        Wh_sb = sp.tile([128, 8 * D], F32, name="Wh_sb", tag="Wh_sb")
        # rhs1: two i-buffers of [128, N]; rows 64:128 hold hT_bf (constant)
        rhs1_sb = sp.tile([128, 2 * N], BF16, name="rhs1_sb", tag="rhs1_sb")
        rhs2_sb = sp.tile([D + 1, 2 * N], F32, name="rhs2_sb", tag="rhs2_sb")
        e_sb = sp.tile([R, N], F32, name="e_sb", tag="e_sb")
        em_sb = sp.tile([R, N], F32, name="em_sb", tag="em_sb")
        ex_sb = sp.tile([R, N], F32, name="ex_sb", tag="ex_sb")
        attn_sb = sp.tile([R, N], F32, name="attn_sb", tag="attn_sb")
        attnT_sb = sp.tile([128, N], F32, name="attnT_sb", tag="attnT_sb")
        scr_sb = sp.tile([1, 8], F32, name="scr_sb", tag="scr_sb")
        red_sb = sp.tile([R, 8], F32, name="red_sb", tag="red_sb")
        hp_sb = sp.tile([R, D], F32, name="hp_sb", tag="hp_sb")
        xm_sb = sp.tile([R, D], F32, name="xm_sb", tag="xm_sb")
        sq_sb = sp.tile([R, D], F32, name="sq_sb", tag="sq_sb")
        o_sb = sp.tile([R, D], F32, name="o_sb", tag="o_sb")

        # ---------------- load inputs ----------------
        nc.sync.dma_start(hT_sb[:], hT_f)
        nc.sync.dma_start(hTr_sb[:], hTr)
        nc.sync.dma_start(hrows_sb[:], hrows)
        nc.sync.dma_start(adjf_sb[:], adjf)
        nc.sync.dma_start(lhsT1_sb[:], lhsT1)
        nc.sync.dma_start(lhsT2u_sb[:], lhsT2u)
        nc.sync.dma_start(Ej_sb[:], Ej)
        nc.sync.dma_start(Ei_sb[:], Ei)
        nc.sync.dma_start(Wm_sb[:], Wm)
        nc.sync.dma_start(Ai_sb[:], Ai)
        nc.sync.dma_start(Ae_sb[:], Ae)
        nc.sync.dma_start(b1_sb[:], b1col)
        nc.sync.dma_start(eb_sb[:], ebcol)
        nc.sync.dma_start(iden_sb[:], iden)
        nc.sync.dma_start(lngr_sb[:], lngr)
        nc.sync.dma_start(lnbr_sb[:], lnbr)
        # hT_bf straight into both rhs1 buffers' lower half (partitions 64:128)
        nc.sync.dma_start(rhs1_sb[D:2 * D, 0:N], hT_bf)
        nc.sync.dma_start(rhs1_sb[D:2 * D, N:2 * N], hT_bf)

        # warm ACT table sets early (exp/ln)
        nc.vector.memset(scr_sb[:], 1.0)
        nc.scalar.activation(scr_sb[0:1, 0:1], scr_sb[0:1, 1:2], AF.Exp)
        nc.scalar.activation(scr_sb[0:1, 2:3], scr_sb[0:1, 3:4], AF.Ln)

        # ---------------- setup math ----------------
        with tc.tile_pool(name="ps_setup", bufs=1, space="PSUM") as psp:
            # ejT (bf16) over all N columns
            for jh in range(2):
                ej_ps = psp.tile([D, 512], F32, name="ej_ps", bufs=2)
                nc.tensor.matmul(ej_ps[:], Ej_sb[:], hT_sb[:, jh * 512:(jh + 1) * 512])
                nc.vector.tensor_copy(ejT_bf_sb[:, jh * 512:(jh + 1) * 512], ej_ps[:])
            # WhTr = W^T-projected rows (feature-major, this core's columns)
            whtr_ps = psp.tile([D, R], F32, name="whtr_ps")
            nc.tensor.matmul(whtr_ps[:], Wm_sb[:], hTr_sb[:])
            nc.vector.tensor_copy(WhTr_sb[:], whtr_ps[:])
            # eibr = E_i^T h_rows + edge_b
            eib_ps = psp.tile([D, R], F32, name="eib_ps")
            nc.tensor.matmul(eib_ps[:], Ei_sb[:], hTr_sb[:])
            nc.vector.tensor_scalar(eibr_sb[:], eib_ps[:], eb_sb[:], None, op0=ALU.add)
            # qb = A_i^T WhTr + b1
            q_ps = psp.tile([D, R], F32, name="q_ps")
            nc.tensor.matmul(q_ps[:], Ai_sb[:], WhTr_sb[:])
            nc.vector.tensor_scalar(qb_sb[:], q_ps[:], b1_sb[:], None, op0=ALU.add)
            # u = qb + 0.2 * A_e^T eibr ; row 64 = +C
            z_ps = psp.tile([D, R], F32, name="z_ps")
            nc.tensor.matmul(z_ps[:], Ae_sb[:], eibr_sb[:])
            nc.vector.scalar_tensor_tensor(
                u_sb[0:D, :], z_ps[:], ALPHA, qb_sb[:], op0=ALU.mult, op1=ALU.add)
            nc.vector.memset(u_sb[D:D + 1, :], C_ROW64)
            # Wh node-major [128, 64] x 8 tiles
            for t in range(8):
                wh_ps = psp.tile([128, D], F32, name="wh_ps", bufs=2)
                nc.tensor.matmul(wh_ps[:], hT_sb[:, t * 128:(t + 1) * 128], Wm_sb[:])
                nc.vector.tensor_copy(Wh_sb[:, t * D:(t + 1) * D], wh_ps[:])

        # ---------------- main loop over this core's 128 rows ----------------
        with tc.tile_pool(name="ps_mm1", bufs=2, space="PSUM") as pmm1, \
             tc.tile_pool(name="ps_e", bufs=4, space="PSUM") as pe:
            bankE = None
            for i in range(R):
                g = i % 32
                grp = i // 32
                buf = i % 2
                if g == 0:
                    bankE = [pe.tile([32, 512], F32, name="bankE", tag="bankE")
                             for _ in range(2)]
                # stage 1: relu(ei + ej + b) into rhs1 upper half
                nc.vector.tensor_scalar(
                    rhs1_sb[0:D, buf * N:(buf + 1) * N],
                    ejT_bf_sb[:],
                    eibr_sb[:, i:i + 1], 0.0, op0=ALU.add, op1=ALU.max)
                # main matmul: psum1[0:64] = t+v ; psum1[64] = 0.2*w2^T(t+v)+C...
                # (C added later via u bias; col64 has no C)
                psum1 = pmm1.tile([D + 1, N], F32, name="psum1", tag="psum1")
                for jh in range(2):
                    nc.tensor.matmul(
                        psum1[:, jh * 512:(jh + 1) * 512],
                        lhsT1_sb[:],
                        rhs1_sb[:, buf * N + jh * 512: buf * N + (jh + 1) * 512])
                # stage 2: rhs2 = relu(psum1 + u)   (row 64: +C keeps it positive)
                r2 = rhs2_sb[:, buf * N:(buf + 1) * N]
                if i % 3 == 0:
                    nc.vector.tensor_scalar(
                        r2, psum1[:], u_sb[:, i:i + 1], 0.0,
                        op0=ALU.add, op1=ALU.max)
                else:
                    nc.scalar.activation(r2, psum1[:], AF.Relu,
                                         bias=u_sb[:, i:i + 1], scale=1.0)
                # score matmul: accumulate e rows into persistent banks
                for jh in range(2):
                    nc.tensor.matmul(
                        bankE[jh][:],
                        lhsT2u_sb[:, g * 32:(g + 1) * 32],
                        rhs2_sb[:, buf * N + jh * 512: buf * N + (jh + 1) * 512],
                        start=(g == 0), stop=(g == 31))
                if g == 31:
                    for jh in range(2):
                        dst = e_sb[grp * 32:(grp + 1) * 32,
                                   jh * 512:(jh + 1) * 512]
                        if (grp + jh) % 2 == 0:
                            nc.vector.tensor_copy(dst, bankE[jh][:])
                        else:
                            nc.scalar.copy(dst, bankE[jh][:])

        # ---------------- mask + softmax ----------------
        nc.vector.tensor_tensor(em_sb[:], e_sb[:], adjf_sb[:], op=ALU.mult)
        nc.vector.reduce_max(red_sb[:, 0:1], em_sb[:], axis=AX.X)
        nc.vector.tensor_scalar(red_sb[:, 1:2], red_sb[:, 0:1], -1.0, None,
                                op0=ALU.mult)
        nc.scalar.activation(ex_sb[:], em_sb[:], AF.Exp,
                             bias=red_sb[:, 1:2], scale=1.0,
                             accum_out=red_sb[:, 2:3])
        nc.vector.reciprocal(red_sb[:, 3:4], red_sb[:, 2:3])
        nc.vector.tensor_scalar(attn_sb[:], ex_sb[:], red_sb[:, 3:4], None,
                                op0=ALU.mult)

        # ---------------- h' = attn @ Wh + h ; LayerNorm ----------------
        with tc.tile_pool(name="ps_fin", bufs=4, space="PSUM") as pf:
            for t in range(8):
                tp_ps = pf.tile([128, 128], F32, name="tp_ps", tag="tp")
                nc.tensor.transpose(tp_ps[:], attn_sb[:, t * 128:(t + 1) * 128],
                                    iden_sb[:])
                nc.vector.tensor_copy(attnT_sb[:, t * 128:(t + 1) * 128], tp_ps[:])
            hp_ps = pf.tile([R, D], F32, name="hp_ps", bufs=1)
            for t in range(8):
                nc.tensor.matmul(hp_ps[:], attnT_sb[:, t * 128:(t + 1) * 128],
                                 Wh_sb[:, t * D:(t + 1) * D],
                                 start=(t == 0), stop=(t == 7))
            nc.vector.tensor_tensor(hp_sb[:], hp_ps[:], hrows_sb[:], op=ALU.add)

        nc.vector.reduce_sum(red_sb[:, 4:5], hp_sb[:], axis=AX.X)
        nc.vector.tensor_scalar(red_sb[:, 5:6], red_sb[:, 4:5], 1.0 / D, None,
                                op0=ALU.mult)
        nc.vector.tensor_scalar(xm_sb[:], hp_sb[:], red_sb[:, 5:6], None,
                                op0=ALU.subtract)
        nc.vector.tensor_tensor(sq_sb[:], xm_sb[:], xm_sb[:], op=ALU.mult)
        nc.vector.reduce_sum(red_sb[:, 6:7], sq_sb[:], axis=AX.X)
        # rstd = exp(-0.5 * ln(var + eps))
        nc.vector.tensor_scalar(red_sb[:, 6:7], red_sb[:, 6:7], 1.0 / D,
                                LN_EPS, op0=ALU.mult, op1=ALU.add)
        nc.scalar.activation(red_sb[:, 7:8], red_sb[:, 6:7], AF.Ln)
        nc.scalar.activation(red_sb[:, 7:8], red_sb[:, 7:8], AF.Exp,
                             bias=0.0, scale=-0.5)
        nc.vector.tensor_scalar(xm_sb[:], xm_sb[:], red_sb[:, 7:8], None,
                                op0=ALU.mult)
        nc.vector.tensor_tensor(o_sb[:], xm_sb[:], lngr_sb[:], op=ALU.mult)
        nc.vector.tensor_tensor(o_sb[:], o_sb[:], lnbr_sb[:], op=ALU.add)
        nc.sync.dma_start(out_d, o_sb[:])

    nc.compile()
    return nc


def _host_prep(inputs):
    h = np.asarray(inputs["h"], np.float32)[0]            # [N, D]
    adj = np.asarray(inputs["adj"])[0]                    # [N, N] int32
    W = np.asarray(inputs["W"], np.float32)
    attn_w1 = np.asarray(inputs["attn_w1"], np.float32)
    attn_b1 = np.asarray(inputs["attn_b1"], np.float32)
    attn_w2 = np.asarray(inputs["attn_w2"], np.float32)
    edge_w = np.asarray(inputs["edge_w"], np.float32)
    edge_b = np.asarray(inputs["edge_b"], np.float32)
    ln_g = np.asarray(inputs["ln_g"], np.float32)
    ln_b = np.asarray(inputs["ln_b"], np.float32)

    A_i, A_j, A_e = attn_w1[:D], attn_w1[D:2 * D], attn_w1[2 * D:]
    E_i, E_j = edge_w[:D], edge_w[D:]
    w2 = attn_w2[:, 0]

    hT = np.ascontiguousarray(h.T)                        # [D, N]
    Mv = W @ A_j + ALPHA * (E_j @ A_e)
    lhsT1 = np.zeros((2 * D, D + 1), np.float32)
    lhsT1[:D, :D] = 0.8 * A_e
    lhsT1[D:, :D] = Mv
    lhsT1[:D, D] = 0.8 * ALPHA * (A_e @ w2)
    lhsT1[D:, D] = ALPHA * (Mv @ w2)
    lhsT2u = np.zeros((D + 1, 32 * 32), np.float32)
    for g in range(32):
        lhsT2u[:D, g * 32 + g] = 0.8 * w2
        lhsT2u[D, g * 32 + g] = 1.0

    rep = {
        "hT_f": hT,
        "hT_bf": hT.astype(ml_dtypes.bfloat16),
        "lhsT1": lhsT1.astype(ml_dtypes.bfloat16),
        "lhsT2u": lhsT2u,
        "Ej": np.ascontiguousarray(E_j),
        "Ei": np.ascontiguousarray(E_i),
        "Wm": W,
        "Ai": np.ascontiguousarray(A_i),
        "Ae": np.ascontiguousarray(A_e),
        "b1col": np.ascontiguousarray(attn_b1[:, None]),
        "ebcol": np.ascontiguousarray(edge_b[:, None]),
        "iden": np.eye(128, dtype=np.float32),
        "lngr": np.broadcast_to(ln_g, (R, D)).copy(),
        "lnbr": np.broadcast_to(ln_b, (R, D)).copy(),
    }
    in_maps = []
    for c in range(NCORES):
        rows = slice(c * R, (c + 1) * R)
        m = dict(rep)
        m["hTr"] = np.ascontiguousarray(hT[:, rows])
        m["hrows"] = np.ascontiguousarray(h[rows])
        m["adjf"] = adj[rows].astype(np.float32)
        in_maps.append(m)
    return in_maps


def _get_nc():
    if "nc" not in _CACHE:
        _CACHE["nc"] = _build_program()
    return _CACHE["nc"]


def kernel(**inputs) -> np.ndarray:
    nc = _get_nc()
    in_maps = _host_prep(inputs)
    res = run_bass_kernel_spmd(nc, in_maps, list(range(NCORES))).results
    out = np.concatenate([res[c]["out"] for c in range(NCORES)], axis=0)
    return out[None].astype(np.float32)



# revision 2
# speedup vs baseline: 2.1663x; 2.1663x over previous
"""Trainium2 Bass kernel for EnhancedGraphAttentionLayer (B=1, N=1024, D=64).

Sharding: destination-node rows split across 8 cores (128 rows each).
Each core is fully independent (no collectives): it holds h replicated and
computes its 128 rows of scores/softmax/attention locally.

v2 decomposition (all-bf16 matmuls):
  LeakyReLU(x) = 0.2*x + 0.8*relu(x) at both nonlinearities.
  pre_ij = u_i + 0.8*A_e^T relu(s_ij) + Mv^T h_j,  s_ij = ei_i + ej_j + eb
  e_ij  = 0.8*sum_k w2_k relu(pre_k) + 0.2*w2^T(t+v) + const_i
  Fold c_k = 0.8*|w2_k| into stage-1 weight cols + u rows (LeakyReLU is
  positively homogeneous), so stage-3 reduces with exact +-1 signs in bf16.
  The 0.2-linear score part splits into:
    slin (from relu(s)): carried as two extra +-slin psum rows
      (relu(x)-relu(-x)=x, exact), reduced with +1/-1 in stage-3;
    jlin (per-j const): host-precomputed row vector, added into the score
      psum bank with one ones-weight matmul pair per 32-row group.
  Mask is additive (-30000 on adj==0), so no relu-safe score shift needed.
"""
import sys
import numpy as np

if "/opt/trn_rl_repo" not in sys.path:
    sys.path.insert(0, "/opt/trn_rl_repo")

import ml_dtypes
import concourse.bass as bass
import concourse.bacc as bacc
import concourse.mybir as mybir
import concourse.tile as tile
from concourse.bass_utils import run_bass_kernel_spmd

F32 = mybir.dt.float32
BF16 = mybir.dt.bfloat16
AF = mybir.ActivationFunctionType
ALU = mybir.AluOpType
AX = mybir.AxisListType

N = 1024
D = 64
NCORES = 8
R = N // NCORES          # 128 rows per core
ALPHA = 0.2
LN_EPS = 1e-5
DK = D + 2               # 64 features + slin+/- rows

_CACHE = {}


def _build_program():
    nc = bacc.Bacc("TRN2", target_bir_lowering=False, debug=False,
                   num_devices=NCORES)

    def din(name, shape, dt):
        return nc.dram_tensor(name, shape, dt, kind="ExternalInput").ap()

    hT_f = din("hT_f", [D, N], F32)
    hT_bf = din("hT_bf", [D, N], BF16)
    hTr = din("hTr", [D, R], F32)
    hrows = din("hrows", [R, D], F32)
    adjm = din("adjm", [R, N], F32)
    lhsT1 = din("lhsT1", [2 * D, DK], BF16)
    lhsT2 = din("lhsT2", [DK, 32 * 32], BF16)
    jones = din("jones", [1, 32], BF16)
    jlinT = din("jlinT", [1, N], BF16)
    Ej = din("Ej", [D, D], F32)
    Ei = din("Ei", [D, D], F32)
    Wm = din("Wm", [D, D], F32)
    Ai = din("Ai", [D, D], F32)      # A_i @ diag(c) (host-scaled)
    Ae = din("Ae", [D, D], F32)      # A_e @ diag(c) (host-scaled)
    b1col = din("b1col", [D, 1], F32)  # c * attn_b1
    ebcol = din("ebcol", [D, 1], F32)
    iden = din("iden", [128, 128], BF16)
    lngr = din("lngr", [R, D], F32)
    lnbr = din("lnbr", [R, D], F32)
    out_d = nc.dram_tensor("out", [R, D], F32, kind="ExternalOutput").ap()

    with tile.TileContext(nc) as tc, \
         tc.tile_pool(name="static", bufs=1) as sp:
        # ---------------- static SBUF tiles ----------------
        hT_sb = sp.tile([D, N], F32, name="hT_sb", tag="hT_sb")
        hTr_sb = sp.tile([D, R], F32, name="hTr_sb", tag="hTr_sb")
        hrows_sb = sp.tile([R, D], F32, name="hrows_sb", tag="hrows_sb")
        adjm_sb = sp.tile([R, N], F32, name="adjm_sb", tag="adjm_sb")
        lhsT1_sb = sp.tile([2 * D, DK], BF16, name="lhsT1_sb", tag="lhsT1_sb")
        lhsT2_sb = sp.tile([DK, 32 * 32], BF16, name="lhsT2_sb", tag="lhsT2_sb")
        jones_sb = sp.tile([1, 32], BF16, name="jones_sb", tag="jones_sb")
        jlinT_sb = sp.tile([1, N], BF16, name="jlinT_sb", tag="jlinT_sb")
        Ej_sb = sp.tile([D, D], F32, name="Ej_sb", tag="Ej_sb")
        Ei_sb = sp.tile([D, D], F32, name="Ei_sb", tag="Ei_sb")
        Wm_sb = sp.tile([D, D], F32, name="Wm_sb", tag="Wm_sb")
        Ai_sb = sp.tile([D, D], F32, name="Ai_sb", tag="Ai_sb")
        Ae_sb = sp.tile([D, D], F32, name="Ae_sb", tag="Ae_sb")
        b1_sb = sp.tile([D, 1], F32, name="b1_sb", tag="b1_sb")
        eb_sb = sp.tile([D, 1], F32, name="eb_sb", tag="eb_sb")
        iden_sb = sp.tile([128, 128], BF16, name="iden_sb", tag="iden_sb")
        lngr_sb = sp.tile([R, D], F32, name="lngr_sb", tag="lngr_sb")
        lnbr_sb = sp.tile([R, D], F32, name="lnbr_sb", tag="lnbr_sb")

        ejT_bf_sb = sp.tile([D, N], BF16, name="ejT_bf_sb", tag="ejT_bf_sb")
        eibr_sb = sp.tile([D, R], F32, name="eibr_sb", tag="eibr_sb")
        WhTr_sb = sp.tile([D, R], F32, name="WhTr_sb", tag="WhTr_sb")
        qb_sb = sp.tile([D, R], F32, name="qb_sb", tag="qb_sb")
        u_sb = sp.tile([DK, R], F32, name="u_sb", tag="u_sb")
        Whb_sb = sp.tile([128, 8 * D], BF16, name="Whb_sb", tag="Whb_sb")
        # rhs1: two i-buffers of [128, N]; rows 64:128 hold hT_bf (constant)
        rhs1_sb = sp.tile([128, 2 * N], BF16, name="rhs1_sb", tag="rhs1_sb")
        rhs2_sb = sp.tile([DK, 2 * N], BF16, name="rhs2_sb", tag="rhs2_sb")
        e_sb = sp.tile([R, N], F32, name="e_sb", tag="e_sb")
        em_sb = sp.tile([R, N], F32, name="em_sb", tag="em_sb")
        ex_sb = sp.tile([R, N], F32, name="ex_sb", tag="ex_sb")
        attn_sb = sp.tile([R, N], BF16, name="attn_sb", tag="attn_sb")
        attnT_sb = sp.tile([128, N], BF16, name="attnT_sb", tag="attnT_sb")
        scr_sb = sp.tile([1, 8], F32, name="scr_sb", tag="scr_sb")
        red_sb = sp.tile([R, 8], F32, name="red_sb", tag="red_sb")
        hp_sb = sp.tile([R, D], F32, name="hp_sb", tag="hp_sb")
        xm_sb = sp.tile([R, D], F32, name="xm_sb", tag="xm_sb")
        o_sb = sp.tile([R, D], F32, name="o_sb", tag="o_sb")

        # ---------------- load inputs ----------------
        nc.sync.dma_start(hT_sb[:], hT_f)
        nc.sync.dma_start(hTr_sb[:], hTr)
        nc.sync.dma_start(hrows_sb[:], hrows)
        nc.sync.dma_start(adjm_sb[:], adjm)
        nc.sync.dma_start(lhsT1_sb[:], lhsT1)
        nc.sync.dma_start(lhsT2_sb[:], lhsT2)
        nc.sync.dma_start(jones_sb[:], jones)
        nc.sync.dma_start(jlinT_sb[:], jlinT)
        nc.sync.dma_start(Ej_sb[:], Ej)
        nc.sync.dma_start(Ei_sb[:], Ei)
        nc.sync.dma_start(Wm_sb[:], Wm)
        nc.sync.dma_start(Ai_sb[:], Ai)
        nc.sync.dma_start(Ae_sb[:], Ae)
        nc.sync.dma_start(b1_sb[:], b1col)
        nc.sync.dma_start(eb_sb[:], ebcol)
        nc.sync.dma_start(iden_sb[:], iden)
        nc.sync.dma_start(lngr_sb[:], lngr)
        nc.sync.dma_start(lnbr_sb[:], lnbr)
        # hT_bf straight into both rhs1 buffers' lower half (partitions 64:128)
        nc.sync.dma_start(rhs1_sb[D:2 * D, 0:N], hT_bf)
        nc.sync.dma_start(rhs1_sb[D:2 * D, N:2 * N], hT_bf)

        # warm ACT table sets early (exp/ln)
        nc.vector.memset(scr_sb[:], 1.0)
        nc.scalar.activation(scr_sb[0:1, 0:1], scr_sb[0:1, 1:2], AF.Exp)
        nc.scalar.activation(scr_sb[0:1, 2:3], scr_sb[0:1, 3:4], AF.Ln)

        # ---------------- setup math ----------------
        with tc.tile_pool(name="ps_setup", bufs=1, space="PSUM") as psp:
            # ejT (bf16) over all N columns
            for jh in range(2):
                ej_ps = psp.tile([D, 512], F32, name="ej_ps", bufs=2)
                nc.tensor.matmul(ej_ps[:], Ej_sb[:], hT_sb[:, jh * 512:(jh + 1) * 512])
                nc.vector.tensor_copy(ejT_bf_sb[:, jh * 512:(jh + 1) * 512], ej_ps[:])
            # WhTr = W^T-projected rows (feature-major, this core's columns)
            whtr_ps = psp.tile([D, R], F32, name="whtr_ps")
            nc.tensor.matmul(whtr_ps[:], Wm_sb[:], hTr_sb[:])
            nc.vector.tensor_copy(WhTr_sb[:], whtr_ps[:])
            # eibr = E_i^T h_rows + edge_b
            eib_ps = psp.tile([D, R], F32, name="eib_ps")
            nc.tensor.matmul(eib_ps[:], Ei_sb[:], hTr_sb[:])
            nc.vector.tensor_scalar(eibr_sb[:], eib_ps[:], eb_sb[:], None, op0=ALU.add)
            # qb = (A_i diag(c))^T WhTr + c*b1
            q_ps = psp.tile([D, R], F32, name="q_ps")
            nc.tensor.matmul(q_ps[:], Ai_sb[:], WhTr_sb[:])
            nc.vector.tensor_scalar(qb_sb[:], q_ps[:], b1_sb[:], None, op0=ALU.add)
            # u[0:64] = qb + ALPHA * (A_e diag(c))^T eibr ; rows 64,65 = 0
            z_ps = psp.tile([D, R], F32, name="z_ps")
            nc.tensor.matmul(z_ps[:], Ae_sb[:], eibr_sb[:])
            nc.vector.scalar_tensor_tensor(
                u_sb[0:D, :], z_ps[:], ALPHA, qb_sb[:], op0=ALU.mult, op1=ALU.add)
            nc.vector.memset(u_sb[D:DK, :], 0.0)
            # Wh node-major [128, 64] x 8 tiles (bf16 for the final matmul)
            for t in range(8):
                wh_ps = psp.tile([128, D], F32, name="wh_ps", bufs=2)
                nc.tensor.matmul(wh_ps[:], hT_sb[:, t * 128:(t + 1) * 128], Wm_sb[:])
                nc.vector.tensor_copy(Whb_sb[:, t * D:(t + 1) * D], wh_ps[:])

        # ---------------- main loop over this core's 128 rows ----------------
        with tc.tile_pool(name="ps_mm1", bufs=2, space="PSUM") as pmm1, \
             tc.tile_pool(name="ps_e", bufs=4, space="PSUM") as pe:
            bankE = None
            for i in range(R):
                g = i % 32
                grp = i // 32
                buf = i % 2
                if g == 0:
                    bankE = [pe.tile([32, 512], F32, name="bankE", tag="bankE")
                             for _ in range(2)]
                # stage 1: relu(ei + ej + b) into rhs1 upper half
                nc.vector.tensor_scalar(
                    rhs1_sb[0:D, buf * N:(buf + 1) * N],
                    ejT_bf_sb[:],
                    eibr_sb[:, i:i + 1], 0.0, op0=ALU.add, op1=ALU.max)
                # main matmul: psum1[0:64] = c*(t+v) ; rows 64,65 = +-slin
                psum1 = pmm1.tile([DK, N], F32, name="psum1", tag="psum1")
                for jh in range(2):
                    nc.tensor.matmul(
                        psum1[:, jh * 512:(jh + 1) * 512],
                        lhsT1_sb[:],
                        rhs1_sb[:, buf * N + jh * 512: buf * N + (jh + 1) * 512])
                # stage 2: rhs2 = relu(psum1 + u)  (bf16)
                r2 = rhs2_sb[:, buf * N:(buf + 1) * N]
                if i % 4 == 0:
                    nc.vector.tensor_scalar(
                        r2, psum1[:], u_sb[:, i:i + 1], 0.0,
                        op0=ALU.add, op1=ALU.max)
                else:
                    nc.scalar.activation(r2, psum1[:], AF.Relu,
                                         bias=u_sb[:, i:i + 1], scale=1.0)
                # score matmul: accumulate e rows into persistent banks
                for jh in range(2):
                    nc.tensor.matmul(
                        bankE[jh][:],
                        lhsT2_sb[:, g * 32:(g + 1) * 32],
                        rhs2_sb[:, buf * N + jh * 512: buf * N + (jh + 1) * 512],
                        start=(g == 0), stop=False)
                if g == 31:
                    # add per-j linear part (jlin) to all 32 rows, close group
                    for jh in range(2):
                        nc.tensor.matmul(
                            bankE[jh][:],
                            jones_sb[:],
                            jlinT_sb[:, jh * 512:(jh + 1) * 512],
                            start=False, stop=True)
                    for jh in range(2):
                        dst = e_sb[grp * 32:(grp + 1) * 32,
                                   jh * 512:(jh + 1) * 512]
                        if (grp + jh) % 2 == 0:
                            nc.vector.tensor_copy(dst, bankE[jh][:])
                        else:
                            nc.scalar.copy(dst, bankE[jh][:])

        # ---------------- mask + softmax ----------------
        nc.vector.tensor_tensor(em_sb[:], e_sb[:], adjm_sb[:], op=ALU.add)
        nc.vector.reduce_max(red_sb[:, 0:1], em_sb[:], axis=AX.X)
        nc.vector.tensor_scalar(red_sb[:, 1:2], red_sb[:, 0:1], -1.0, None,
                                op0=ALU.mult)
        nc.scalar.activation(ex_sb[:], em_sb[:], AF.Exp,
                             bias=red_sb[:, 1:2], scale=1.0,
                             accum_out=red_sb[:, 2:3])
        nc.vector.reciprocal(red_sb[:, 3:4], red_sb[:, 2:3])
        nc.vector.tensor_scalar(attn_sb[:], ex_sb[:], red_sb[:, 3:4], None,
                                op0=ALU.mult)

        # ---------------- h' = attn @ Wh + h ; LayerNorm ----------------
        with tc.tile_pool(name="ps_fin", bufs=4, space="PSUM") as pf:
            for t in range(8):
                tp_ps = pf.tile([128, 128], BF16, name="tp_ps", tag="tp")
                nc.tensor.transpose(tp_ps[:], attn_sb[:, t * 128:(t + 1) * 128],
                                    iden_sb[:])
                nc.vector.tensor_copy(attnT_sb[:, t * 128:(t + 1) * 128], tp_ps[:])
            hp_ps = pf.tile([R, D], F32, name="hp_ps", bufs=1)
            for t in range(8):
                nc.tensor.matmul(hp_ps[:], attnT_sb[:, t * 128:(t + 1) * 128],
                                 Whb_sb[:, t * D:(t + 1) * D],
                                 start=(t == 0), stop=(t == 7))
            nc.vector.tensor_tensor(hp_sb[:], hp_ps[:], hrows_sb[:], op=ALU.add)

        nc.vector.reduce_sum(red_sb[:, 4:5], hp_sb[:], axis=AX.X)
        nc.vector.tensor_scalar(red_sb[:, 5:6], red_sb[:, 4:5], 1.0 / D, None,
                                op0=ALU.mult)
        nc.vector.tensor_scalar(xm_sb[:], hp_sb[:], red_sb[:, 5:6], None,
                                op0=ALU.subtract)
        nc.vector.tensor_tensor(o_sb[:], xm_sb[:], xm_sb[:], op=ALU.mult)
        nc.vector.reduce_sum(red_sb[:, 6:7], o_sb[:], axis=AX.X)
        # rstd = exp(-0.5 * ln(var + eps))
        nc.vector.tensor_scalar(red_sb[:, 6:7], red_sb[:, 6:7], 1.0 / D,
                                LN_EPS, op0=ALU.mult, op1=ALU.add)
        nc.scalar.activation(red_sb[:, 7:8], red_sb[:, 6:7], AF.Ln)
        nc.scalar.activation(red_sb[:, 7:8], red_sb[:, 7:8], AF.Exp,
                             bias=0.0, scale=-0.5)
        nc.vector.tensor_scalar(xm_sb[:], xm_sb[:], red_sb[:, 7:8], None,
                                op0=ALU.mult)
        nc.vector.tensor_tensor(o_sb[:], xm_sb[:], lngr_sb[:], op=ALU.mult)
        nc.vector.tensor_tensor(o_sb[:], o_sb[:], lnbr_sb[:], op=ALU.add)
        nc.sync.dma_start(out_d, o_sb[:])

    nc.compile()
    return nc


def _host_prep(inputs):
    h = np.asarray(inputs["h"], np.float32)[0]            # [N, D]
    adj = np.asarray(inputs["adj"])[0]                    # [N, N] int32
    W = np.asarray(inputs["W"], np.float32)
    attn_w1 = np.asarray(inputs["attn_w1"], np.float32)
    attn_b1 = np.asarray(inputs["attn_b1"], np.float32)
    attn_w2 = np.asarray(inputs["attn_w2"], np.float32)
    edge_w = np.asarray(inputs["edge_w"], np.float32)
    edge_b = np.asarray(inputs["edge_b"], np.float32)
    ln_g = np.asarray(inputs["ln_g"], np.float32)
    ln_b = np.asarray(inputs["ln_b"], np.float32)

    A_i, A_j, A_e = attn_w1[:D], attn_w1[D:2 * D], attn_w1[2 * D:]
    E_i, E_j = edge_w[:D], edge_w[D:]
    w2 = attn_w2[:, 0]

    hT = np.ascontiguousarray(h.T)                        # [D, N]
    Mv = W @ A_j + ALPHA * (E_j @ A_e)
    c = 0.8 * np.abs(w2)
    sgn = np.sign(w2).astype(np.float32)

    lhsT1 = np.zeros((2 * D, DK), np.float32)
    lhsT1[:D, :D] = 0.8 * A_e * c[None, :]
    lhsT1[D:, :D] = Mv * c[None, :]
    slw = 0.8 * ALPHA * (A_e @ w2)
    lhsT1[:D, D] = slw
    lhsT1[:D, D + 1] = -slw

    lhsT2 = np.zeros((DK, 32 * 32), np.float32)
    for g in range(32):
        lhsT2[:D, g * 32 + g] = sgn
        lhsT2[D, g * 32 + g] = 1.0
        lhsT2[D + 1, g * 32 + g] = -1.0

    jlin = ALPHA * (h @ (Mv @ w2))                        # [N]

    rep = {
        "hT_f": hT,
        "hT_bf": hT.astype(ml_dtypes.bfloat16),
        "lhsT1": lhsT1.astype(ml_dtypes.bfloat16),
        "lhsT2": lhsT2.astype(ml_dtypes.bfloat16),
        "jones": np.ones((1, 32), ml_dtypes.bfloat16),
        "jlinT": jlin[None, :].astype(ml_dtypes.bfloat16),
        "Ej": np.ascontiguousarray(E_j),
        "Ei": np.ascontiguousarray(E_i),
        "Wm": W,
        "Ai": np.ascontiguousarray(A_i * c[None, :]),
        "Ae": np.ascontiguousarray(A_e * c[None, :]),
        "b1col": np.ascontiguousarray((c * attn_b1)[:, None]),
        "ebcol": np.ascontiguousarray(edge_b[:, None]),
        "iden": np.eye(128, dtype=ml_dtypes.bfloat16),
        "lngr": np.broadcast_to(ln_g, (R, D)).copy(),
        "lnbr": np.broadcast_to(ln_b, (R, D)).copy(),
    }
    in_maps = []
    for cid in range(NCORES):
        rows = slice(cid * R, (cid + 1) * R)
        m = dict(rep)
        m["hTr"] = np.ascontiguousarray(hT[:, rows])
        m["hrows"] = np.ascontiguousarray(h[rows])
        m["adjm"] = np.where(adj[rows] == 0, np.float32(-30000.0),
                             np.float32(0.0))
        in_maps.append(m)
    return in_maps


def _get_nc():
    if "nc" not in _CACHE:
        _CACHE["nc"] = _build_program()
    return _CACHE["nc"]


def kernel(**inputs) -> np.ndarray:
    nc = _get_nc()
    in_maps = _host_prep(inputs)
    res = run_bass_kernel_spmd(nc, in_maps, list(range(NCORES))).results
    out = np.concatenate([res[c]["out"] for c in range(NCORES)], axis=0)
    return out[None].astype(np.float32)


# revision 8
# speedup vs baseline: 3.2644x; 1.5069x over previous
"""Trainium2 Bass kernel for EnhancedGraphAttentionLayer (B=1, N=1024, D=64).

Sharding: destination-node rows split across 8 cores (128 rows each).
Each core is fully independent (no collectives): it holds h replicated and
computes its 128 rows of scores/softmax/attention locally.

v2 decomposition (all-bf16 matmuls):
  LeakyReLU(x) = 0.2*x + 0.8*relu(x) at both nonlinearities.
  pre_ij = u_i + 0.8*A_e^T relu(s_ij) + Mv^T h_j,  s_ij = ei_i + ej_j + eb
  e_ij  = 0.8*sum_k w2_k relu(pre_k) + 0.2*w2^T(t+v) + const_i
  Fold c_k = 0.8*|w2_k| into stage-1 weight cols + u rows (LeakyReLU is
  positively homogeneous), so stage-3 reduces with exact +-1 signs in bf16.
  The 0.2-linear score part splits into:
    slin (from relu(s)): carried as two extra +-slin psum rows
      (relu(x)-relu(-x)=x, exact), reduced with +1/-1 in stage-3;
    jlin (per-j const): host-precomputed row vector, added into the score
      psum bank with one ones-weight matmul pair per 32-row group.
  Mask is additive (-30000 on adj==0), so no relu-safe score shift needed.
"""
import sys
import numpy as np

if "/opt/trn_rl_repo" not in sys.path:
    sys.path.insert(0, "/opt/trn_rl_repo")

import ml_dtypes
import concourse.bass as bass
import concourse.bacc as bacc
import concourse.mybir as mybir
import concourse.tile as tile
from concourse.bass_utils import run_bass_kernel_spmd

F32 = mybir.dt.float32
BF16 = mybir.dt.bfloat16
AF = mybir.ActivationFunctionType
ALU = mybir.AluOpType
AX = mybir.AxisListType

N = 1024
D = 64
NCORES = 8
R = N // NCORES          # 128 rows per core
ALPHA = 0.2
LN_EPS = 1e-5
DK = D + 2               # 64 features + slin+/- rows

_CACHE = {}


def _build_program():
    nc = bacc.Bacc("TRN2", target_bir_lowering=False, debug=False,
                   num_devices=NCORES)

    def din(name, shape, dt):
        return nc.dram_tensor(name, shape, dt, kind="ExternalInput").ap()

    hT_f = din("hT_f", [D, N], F32)
    hT_bf = din("hT_bf", [D, N], BF16)
    hTr = din("hTr", [D, R], F32)
    hrows = din("hrows", [R, D], F32)
    adjm = din("adjm", [R, N], F32)
    lhsT1 = din("lhsT1", [2 * D, DK], BF16)
    # stage-3 weights padded to the same [128, 66] shape as lhsT1 so the PE
    # never switches weight tile shape (shape alternation blocks HAM warm-up)
    lhsT2 = din("lhsT2", [128, 32 * 128], BF16)
    jones = din("jones", [128, DK], BF16)
    jlinT = din("jlinT", [128, N], BF16)
    Ej = din("Ej", [D, D], F32)
    Ei = din("Ei", [D, D], F32)
    Wm = din("Wm", [D, D], F32)
    Ai = din("Ai", [D, D], F32)      # A_i @ diag(c) (host-scaled)
    Ae = din("Ae", [D, D], F32)      # A_e @ diag(c) (host-scaled)
    b1col = din("b1col", [D, 1], F32)  # c * attn_b1
    ebcol = din("ebcol", [D, 1], F32)
    iden = din("iden", [128, 128], BF16)
    lngr = din("lngr", [R, D], F32)
    lnbr = din("lnbr", [R, D], F32)
    out_d = nc.dram_tensor("out", [R, D], F32, kind="ExternalOutput").ap()

    with tile.TileContext(nc) as tc, \
         tc.tile_pool(name="static", bufs=1) as sp:
        # ---------------- static SBUF tiles ----------------
        hT_sb = sp.tile([D, N], F32, name="hT_sb", tag="hT_sb")
        hTr_sb = sp.tile([D, R], F32, name="hTr_sb", tag="hTr_sb")
        hrows_sb = sp.tile([R, D], F32, name="hrows_sb", tag="hrows_sb")
        adjm_sb = sp.tile([R, N], F32, name="adjm_sb", tag="adjm_sb")
        lhsT1_sb = sp.tile([2 * D, DK], BF16, name="lhsT1_sb", tag="lhsT1_sb")
        lhsT2_sb = sp.tile([128, 32 * 128], BF16, name="lhsT2_sb", tag="lhsT2_sb")
        jones_sb = sp.tile([128, DK], BF16, name="jones_sb", tag="jones_sb")
        jlinT_sb = sp.tile([128, N], BF16, name="jlinT_sb", tag="jlinT_sb")
        Ej_sb = sp.tile([D, D], F32, name="Ej_sb", tag="Ej_sb")
        Ei_sb = sp.tile([D, D], F32, name="Ei_sb", tag="Ei_sb")
        Wm_sb = sp.tile([D, D], F32, name="Wm_sb", tag="Wm_sb")
        Ai_sb = sp.tile([D, D], F32, name="Ai_sb", tag="Ai_sb")
        Ae_sb = sp.tile([D, D], F32, name="Ae_sb", tag="Ae_sb")
        b1_sb = sp.tile([D, 1], F32, name="b1_sb", tag="b1_sb")
        eb_sb = sp.tile([D, 1], F32, name="eb_sb", tag="eb_sb")
        iden_sb = sp.tile([128, 128], BF16, name="iden_sb", tag="iden_sb")
        lngr_sb = sp.tile([R, D], F32, name="lngr_sb", tag="lngr_sb")
        lnbr_sb = sp.tile([R, D], F32, name="lnbr_sb", tag="lnbr_sb")

        ejT_bf_sb = sp.tile([D, N], BF16, name="ejT_bf_sb", tag="ejT_bf_sb")
        eibr_sb = sp.tile([D, R], F32, name="eibr_sb", tag="eibr_sb")
        WhTr_sb = sp.tile([D, R], F32, name="WhTr_sb", tag="WhTr_sb")
        qb_sb = sp.tile([D, R], F32, name="qb_sb", tag="qb_sb")
        u_sb = sp.tile([DK, R], F32, name="u_sb", tag="u_sb")
        Whb_sb = sp.tile([128, 8 * D], BF16, name="Whb_sb", tag="Whb_sb")
        # rhs1: two i-buffers of [128, N]; rows 64:128 hold hT_bf (constant)
        rhs1_sb = sp.tile([128, 2 * N], BF16, name="rhs1_sb", tag="rhs1_sb")
        rhs2_sb = sp.tile([128, 2 * N], BF16, name="rhs2_sb", tag="rhs2_sb")
        e_sb = sp.tile([R, N], F32, name="e_sb", tag="e_sb")
        em_sb = sp.tile([R, N], F32, name="em_sb", tag="em_sb")
        ex_sb = sp.tile([R, N], F32, name="ex_sb", tag="ex_sb")
        attn_sb = sp.tile([R, N], BF16, name="attn_sb", tag="attn_sb")
        attnT_sb = sp.tile([128, N], BF16, name="attnT_sb", tag="attnT_sb")
        scr_sb = sp.tile([1, 8], F32, name="scr_sb", tag="scr_sb")
        red_sb = sp.tile([R, 8], F32, name="red_sb", tag="red_sb")
        hp_sb = sp.tile([R, D], F32, name="hp_sb", tag="hp_sb")
        xm_sb = sp.tile([R, D], F32, name="xm_sb", tag="xm_sb")
        o_sb = sp.tile([R, D], F32, name="o_sb", tag="o_sb")

        # ---------------- load inputs ----------------
        nc.sync.dma_start(hT_sb[:], hT_f)
        nc.sync.dma_start(Ej_sb[:], Ej)
        nc.sync.dma_start(hTr_sb[:], hTr)
        nc.sync.dma_start(hrows_sb[:], hrows)
        nc.sync.dma_start(lhsT1_sb[:], lhsT1)
        nc.sync.dma_start(lhsT2_sb[:], lhsT2)
        nc.sync.dma_start(jones_sb[:], jones)
        nc.sync.dma_start(jlinT_sb[:], jlinT)
        nc.sync.dma_start(Ei_sb[:], Ei)
        nc.sync.dma_start(Wm_sb[:], Wm)
        nc.sync.dma_start(Ai_sb[:], Ai)
        nc.sync.dma_start(Ae_sb[:], Ae)
        nc.sync.dma_start(b1_sb[:], b1col)
        nc.sync.dma_start(eb_sb[:], ebcol)
        nc.sync.dma_start(iden_sb[:], iden)
        nc.sync.dma_start(lngr_sb[:], lngr)
        nc.sync.dma_start(lnbr_sb[:], lnbr)
        nc.sync.dma_start(adjm_sb[:], adjm)
        # hT_bf straight into both rhs1 buffers' lower half (partitions 64:128)
        nc.sync.dma_start(rhs1_sb[D:2 * D, 0:N], hT_bf)
        nc.sync.dma_start(rhs1_sb[D:2 * D, N:2 * N], hT_bf)

        # zero the stage-3 rhs pad rows once (zero weights x junk = NaN risk)
        nc.vector.memset(rhs2_sb[D:128, :], 0.0)

        # warm ACT table sets early (exp/ln)
        nc.vector.memset(scr_sb[:], 1.0)
        nc.scalar.activation(scr_sb[0:1, 0:1], scr_sb[0:1, 1:2], AF.Exp)
        nc.scalar.activation(scr_sb[0:1, 2:3], scr_sb[0:1, 3:4], AF.Ln)

        # ---------------- setup math ----------------
        with tc.tile_pool(name="ps_setup", bufs=1, space="PSUM") as psp:
            # ejT (bf16) over all N columns
            for jh in range(2):
                ej_ps = psp.tile([D, 512], F32, name="ej_ps", bufs=2)
                nc.tensor.matmul(ej_ps[:], Ej_sb[:], hT_sb[:, jh * 512:(jh + 1) * 512])
                nc.vector.tensor_copy(ejT_bf_sb[:, jh * 512:(jh + 1) * 512], ej_ps[:])
            # WhTr = W^T-projected rows (feature-major, this core's columns)
            whtr_ps = psp.tile([D, R], F32, name="whtr_ps")
            nc.tensor.matmul(whtr_ps[:], Wm_sb[:], hTr_sb[:])
            nc.vector.tensor_copy(WhTr_sb[:], whtr_ps[:])
            # eibr = E_i^T h_rows + edge_b
            eib_ps = psp.tile([D, R], F32, name="eib_ps")
            nc.tensor.matmul(eib_ps[:], Ei_sb[:], hTr_sb[:])
            nc.vector.tensor_scalar(eibr_sb[:], eib_ps[:], eb_sb[:], None, op0=ALU.add)
            # qb = (A_i diag(c))^T WhTr + c*b1
            q_ps = psp.tile([D, R], F32, name="q_ps")
            nc.tensor.matmul(q_ps[:], Ai_sb[:], WhTr_sb[:])
            nc.vector.tensor_scalar(qb_sb[:], q_ps[:], b1_sb[:], None, op0=ALU.add)
            # u[0:64] = qb + ALPHA * (A_e diag(c))^T eibr ; rows 64,65 = 0
            z_ps = psp.tile([D, R], F32, name="z_ps")
            nc.tensor.matmul(z_ps[:], Ae_sb[:], eibr_sb[:])
            nc.vector.scalar_tensor_tensor(
                u_sb[0:D, :], z_ps[:], ALPHA, qb_sb[:], op0=ALU.mult, op1=ALU.add)
            nc.vector.memset(u_sb[D:DK, :], 0.0)
            # Wh node-major [128, 64] x 8 tiles (bf16 for the final matmul)
            for t in range(8):
                wh_ps = psp.tile([128, D], F32, name="wh_ps", bufs=2)
                nc.tensor.matmul(wh_ps[:], hT_sb[:, t * 128:(t + 1) * 128], Wm_sb[:])
                nc.vector.tensor_copy(Whb_sb[:, t * D:(t + 1) * D], wh_ps[:])

        # ---------------- main loop over this core's 128 rows ----------------
        with tc.tile_pool(name="ps_mm1", bufs=2, space="PSUM") as pmm1, \
             tc.tile_pool(name="ps_e", bufs=4, space="PSUM") as pe:
            bankE = None
            for i in range(R):
                g = i % 32
                grp = i // 32
                buf = i % 2
                if g == 0:
                    bankE = [pe.tile([DK, 512], F32, name="bankE", tag="bankE")
                             for _ in range(2)]
                # stage 1: relu(ei + ej + b) into rhs1 upper half
                nc.vector.tensor_scalar(
                    rhs1_sb[0:D, buf * N:(buf + 1) * N],
                    ejT_bf_sb[:],
                    eibr_sb[:, i:i + 1], 0.0, op0=ALU.add, op1=ALU.max)
                # main matmul: psum1[0:64] = c*(t+v) ; rows 64,65 = +-slin
                psum1 = pmm1.tile([DK, N], F32, name="psum1", tag="psum1")
                for jh in range(2):
                    nc.tensor.matmul(
                        psum1[:, jh * 512:(jh + 1) * 512],
                        lhsT1_sb[:],
                        rhs1_sb[:, buf * N + jh * 512: buf * N + (jh + 1) * 512])
                # stage 2: rhs2 = relu(psum1 + u)  (bf16)
                r2 = rhs2_sb[0:DK, buf * N:(buf + 1) * N]
                if i % 4 == 0:
                    nc.vector.tensor_scalar(
                        r2, psum1[:], u_sb[:, i:i + 1], 0.0,
                        op0=ALU.add, op1=ALU.max)
                else:
                    nc.scalar.activation(r2, psum1[:], AF.Relu,
                                         bias=u_sb[:, i:i + 1], scale=1.0)
                # score matmul: accumulate e rows into persistent banks
                for jh in range(2):
                    nc.tensor.matmul(
                        bankE[jh][:],
                        lhsT2_sb[:, g * 128:g * 128 + DK],
                        rhs2_sb[:, buf * N + jh * 512: buf * N + (jh + 1) * 512],
                        start=(g == 0), stop=False)
                if g == 31:
                    # add per-j linear part (jlin) to all 32 rows, close group
                    for jh in range(2):
                        nc.tensor.matmul(
                            bankE[jh][:],
                            jones_sb[:],
                            jlinT_sb[:, jh * 512:(jh + 1) * 512],
                            start=False, stop=True)
                    for jh in range(2):
                        dst = e_sb[grp * 32:(grp + 1) * 32,
                                   jh * 512:(jh + 1) * 512]
                        if (grp + jh) % 2 == 0:
                            nc.vector.tensor_copy(dst, bankE[jh][0:32, :])
                        else:
                            nc.scalar.copy(dst, bankE[jh][0:32, :])

        # ---------------- mask + softmax ----------------
        nc.vector.tensor_tensor(em_sb[:], e_sb[:], adjm_sb[:], op=ALU.add)
        nc.vector.reduce_max(red_sb[:, 0:1], em_sb[:], axis=AX.X)
        nc.vector.tensor_scalar(red_sb[:, 1:2], red_sb[:, 0:1], -1.0, None,
                                op0=ALU.mult)
        nc.scalar.activation(ex_sb[:], em_sb[:], AF.Exp,
                             bias=red_sb[:, 1:2], scale=1.0,
                             accum_out=red_sb[:, 2:3])
        nc.vector.reciprocal(red_sb[:, 3:4], red_sb[:, 2:3])
        nc.vector.tensor_scalar(attn_sb[:], ex_sb[:], red_sb[:, 3:4], None,
                                op0=ALU.mult)

        # ---------------- h' = attn @ Wh + h ; LayerNorm ----------------
        with tc.tile_pool(name="ps_fin", bufs=4, space="PSUM") as pf:
            for t in range(8):
                tp_ps = pf.tile([128, 128], BF16, name="tp_ps", tag="tp")
                nc.tensor.transpose(tp_ps[:], attn_sb[:, t * 128:(t + 1) * 128],
                                    iden_sb[:])
                nc.vector.tensor_copy(attnT_sb[:, t * 128:(t + 1) * 128], tp_ps[:])
            hp_ps = pf.tile([R, D], F32, name="hp_ps", bufs=1)
            for t in range(8):
                nc.tensor.matmul(hp_ps[:], attnT_sb[:, t * 128:(t + 1) * 128],
                                 Whb_sb[:, t * D:(t + 1) * D],
                                 start=(t == 0), stop=(t == 7))
            nc.vector.tensor_tensor(hp_sb[:], hp_ps[:], hrows_sb[:], op=ALU.add)

        nc.vector.reduce_sum(red_sb[:, 4:5], hp_sb[:], axis=AX.X)
        nc.vector.tensor_scalar(red_sb[:, 5:6], red_sb[:, 4:5], 1.0 / D, None,
                                op0=ALU.mult)
        nc.vector.tensor_scalar(xm_sb[:], hp_sb[:], red_sb[:, 5:6], None,
                                op0=ALU.subtract)
        nc.vector.tensor_tensor(o_sb[:], xm_sb[:], xm_sb[:], op=ALU.mult)
        nc.vector.reduce_sum(red_sb[:, 6:7], o_sb[:], axis=AX.X)
        # rstd = exp(-0.5 * ln(var + eps))
        nc.vector.tensor_scalar(red_sb[:, 6:7], red_sb[:, 6:7], 1.0 / D,
                                LN_EPS, op0=ALU.mult, op1=ALU.add)
        nc.scalar.activation(red_sb[:, 7:8], red_sb[:, 6:7], AF.Ln)
        nc.scalar.activation(red_sb[:, 7:8], red_sb[:, 7:8], AF.Exp,
                             bias=0.0, scale=-0.5)
        nc.vector.tensor_scalar(xm_sb[:], xm_sb[:], red_sb[:, 7:8], None,
                                op0=ALU.mult)
        nc.vector.tensor_tensor(o_sb[:], xm_sb[:], lngr_sb[:], op=ALU.mult)
        nc.vector.tensor_tensor(o_sb[:], o_sb[:], lnbr_sb[:], op=ALU.add)
        nc.sync.dma_start(out_d, o_sb[:])

    nc.compile()
    return nc


def _host_prep(inputs):
    h = np.asarray(inputs["h"], np.float32)[0]            # [N, D]
    adj = np.asarray(inputs["adj"])[0]                    # [N, N] int32
    W = np.asarray(inputs["W"], np.float32)
    attn_w1 = np.asarray(inputs["attn_w1"], np.float32)
    attn_b1 = np.asarray(inputs["attn_b1"], np.float32)
    attn_w2 = np.asarray(inputs["attn_w2"], np.float32)
    edge_w = np.asarray(inputs["edge_w"], np.float32)
    edge_b = np.asarray(inputs["edge_b"], np.float32)
    ln_g = np.asarray(inputs["ln_g"], np.float32)
    ln_b = np.asarray(inputs["ln_b"], np.float32)

    A_i, A_j, A_e = attn_w1[:D], attn_w1[D:2 * D], attn_w1[2 * D:]
    E_i, E_j = edge_w[:D], edge_w[D:]
    w2 = attn_w2[:, 0]

    hT = np.ascontiguousarray(h.T)                        # [D, N]
    Mv = W @ A_j + ALPHA * (E_j @ A_e)
    c = 0.8 * np.abs(w2)
    sgn = np.sign(w2).astype(np.float32)

    lhsT1 = np.zeros((2 * D, DK), np.float32)
    lhsT1[:D, :D] = 0.8 * A_e * c[None, :]
    lhsT1[D:, :D] = Mv * c[None, :]
    slw = 0.8 * ALPHA * (A_e @ w2)
    lhsT1[:D, D] = slw
    lhsT1[:D, D + 1] = -slw

    lhsT2 = np.zeros((128, 32 * 128), np.float32)
    for g in range(32):
        lhsT2[:D, g * 128 + g] = sgn
        lhsT2[D, g * 128 + g] = 1.0
        lhsT2[D + 1, g * 128 + g] = -1.0

    jones = np.zeros((128, DK), np.float32)
    jones[0, 0:32] = 1.0

    jlin = ALPHA * (h @ (Mv @ w2))                        # [N]
    jlinT = np.zeros((128, N), np.float32)
    jlinT[0] = jlin

    rep = {
        "hT_f": hT,
        "hT_bf": hT.astype(ml_dtypes.bfloat16),
        "lhsT1": lhsT1.astype(ml_dtypes.bfloat16),
        "lhsT2": lhsT2.astype(ml_dtypes.bfloat16),
        "jones": jones.astype(ml_dtypes.bfloat16),
        "jlinT": jlinT.astype(ml_dtypes.bfloat16),
        "Ej": np.ascontiguousarray(E_j),
        "Ei": np.ascontiguousarray(E_i),
        "Wm": W,
        "Ai": np.ascontiguousarray(A_i * c[None, :]),
        "Ae": np.ascontiguousarray(A_e * c[None, :]),
        "b1col": np.ascontiguousarray((c * attn_b1)[:, None]),
        "ebcol": np.ascontiguousarray(edge_b[:, None]),
        "iden": np.eye(128, dtype=ml_dtypes.bfloat16),
        "lngr": np.broadcast_to(ln_g, (R, D)).copy(),
        "lnbr": np.broadcast_to(ln_b, (R, D)).copy(),
    }
    in_maps = []
    for cid in range(NCORES):
        rows = slice(cid * R, (cid + 1) * R)
        m = dict(rep)
        m["hTr"] = np.ascontiguousarray(hT[:, rows])
        m["hrows"] = np.ascontiguousarray(h[rows])
        m["adjm"] = np.where(adj[rows] == 0, np.float32(-30000.0),
                             np.float32(0.0))
        in_maps.append(m)
    return in_maps


def _get_nc():
    if "nc" not in _CACHE:
        _CACHE["nc"] = _build_program()
    return _CACHE["nc"]


def kernel(**inputs) -> np.ndarray:
    nc = _get_nc()
    in_maps = _host_prep(inputs)
    res = run_bass_kernel_spmd(nc, in_maps, list(range(NCORES))).results
    out = np.concatenate([res[c]["out"] for c in range(NCORES)], axis=0)
    return out[None].astype(np.float32)


# revision 14
# speedup vs baseline: 3.8420x; 1.1769x over previous
"""Trainium2 Bass kernel for EnhancedGraphAttentionLayer (B=1, N=1024, D=64).

Sharding: destination-node rows split across 8 cores (128 rows each).
Each core is fully independent (no collectives): it holds h replicated and
computes its 128 rows of scores/softmax/attention locally.

v2 decomposition (all-bf16 matmuls):
  LeakyReLU(x) = 0.2*x + 0.8*relu(x) at both nonlinearities.
  pre_ij = u_i + 0.8*A_e^T relu(s_ij) + Mv^T h_j,  s_ij = ei_i + ej_j + eb
  e_ij  = 0.8*sum_k w2_k relu(pre_k) + 0.2*w2^T(t+v) + const_i
  Fold c_k = 0.8*|w2_k| into stage-1 weight cols + u rows (LeakyReLU is
  positively homogeneous), so stage-3 reduces with exact +-1 signs in bf16.
  The 0.2-linear score part splits into:
    slin (from relu(s)): carried as two extra +-slin psum rows
      (relu(x)-relu(-x)=x, exact), reduced with +1/-1 in stage-3;
    jlin (per-j const): host-precomputed row vector, added into the score
      psum bank with one ones-weight matmul pair per 32-row group.
  Mask is additive (-30000 on adj==0), so no relu-safe score shift needed.
"""
import sys
import numpy as np

if "/opt/trn_rl_repo" not in sys.path:
    sys.path.insert(0, "/opt/trn_rl_repo")

import ml_dtypes
import concourse.bass as bass
import concourse.bacc as bacc
import concourse.mybir as mybir
import concourse.tile as tile
from concourse.bass_utils import run_bass_kernel_spmd

F32 = mybir.dt.float32
BF16 = mybir.dt.bfloat16
AF = mybir.ActivationFunctionType
ALU = mybir.AluOpType
AX = mybir.AxisListType

N = 1024
D = 64
NCORES = 8
R = N // NCORES          # 128 rows per core
ALPHA = 0.2
LN_EPS = 1e-5
DK = D + 2               # 64 features + slin+/- rows

_CACHE = {}


def _build_program():
    nc = bacc.Bacc("TRN2", target_bir_lowering=False, debug=False,
                   num_devices=NCORES)

    def din(name, shape, dt):
        return nc.dram_tensor(name, shape, dt, kind="ExternalInput").ap()

    hT_f = din("hT_f", [D, N], F32)
    hT_bf = din("hT_bf", [D, N], BF16)
    hTr = din("hTr", [D, R], F32)
    hrows = din("hrows", [R, D], F32)
    adjm = din("adjm", [R, N], F32)
    lhsT1 = din("lhsT1", [2 * D, DK], BF16)
    # stage-3 weights padded to the same [128, 66] shape as lhsT1 so the PE
    # never switches weight tile shape (shape alternation blocks HAM warm-up)
    lhsT2 = din("lhsT2", [128, 32 * 128], BF16)
    jones = din("jones", [128, DK], BF16)
    jlinT = din("jlinT", [128, N], BF16)
    Ej = din("Ej", [D, D], BF16)
    Ei = din("Ei", [D, D], F32)
    Wm = din("Wm", [D, D], F32)
    Ai = din("Ai", [D, D], F32)      # A_i @ diag(c) (host-scaled)
    Ae = din("Ae", [D, D], F32)      # A_e @ diag(c) (host-scaled)
    b1col = din("b1col", [D, 1], F32)  # c * attn_b1
    ebcol = din("ebcol", [D, 1], F32)
    iden = din("iden", [128, 128], BF16)
    lngr = din("lngr", [R, D], F32)
    lnbr = din("lnbr", [R, D], F32)
    out_d = nc.dram_tensor("out", [R, D], F32, kind="ExternalOutput").ap()

    with tile.TileContext(nc) as tc, \
         tc.tile_pool(name="static", bufs=1) as sp:
        # ---------------- static SBUF tiles ----------------
        hT_sb = sp.tile([D, N], F32, name="hT_sb", tag="hT_sb")
        hTr_sb = sp.tile([D, R], F32, name="hTr_sb", tag="hTr_sb")
        hrows_sb = sp.tile([R, D], F32, name="hrows_sb", tag="hrows_sb")
        adjm_sb = sp.tile([R, N], F32, name="adjm_sb", tag="adjm_sb")
        lhsT1_sb = sp.tile([2 * D, DK], BF16, name="lhsT1_sb", tag="lhsT1_sb")
        lhsT2_sb = sp.tile([128, 32 * 128], BF16, name="lhsT2_sb", tag="lhsT2_sb")
        jones_sb = sp.tile([128, DK], BF16, name="jones_sb", tag="jones_sb")
        jlinT_sb = sp.tile([128, N], BF16, name="jlinT_sb", tag="jlinT_sb")
        Ej_sb = sp.tile([D, D], BF16, name="Ej_sb", tag="Ej_sb")
        Ei_sb = sp.tile([D, D], F32, name="Ei_sb", tag="Ei_sb")
        Wm_sb = sp.tile([D, D], F32, name="Wm_sb", tag="Wm_sb")
        Ai_sb = sp.tile([D, D], F32, name="Ai_sb", tag="Ai_sb")
        Ae_sb = sp.tile([D, D], F32, name="Ae_sb", tag="Ae_sb")
        b1_sb = sp.tile([D, 1], F32, name="b1_sb", tag="b1_sb")
        eb_sb = sp.tile([D, 1], F32, name="eb_sb", tag="eb_sb")
        iden_sb = sp.tile([128, 128], BF16, name="iden_sb", tag="iden_sb")
        lngr_sb = sp.tile([R, D], F32, name="lngr_sb", tag="lngr_sb")
        lnbr_sb = sp.tile([R, D], F32, name="lnbr_sb", tag="lnbr_sb")

        ejT_bf_sb = sp.tile([D, N], BF16, name="ejT_bf_sb", tag="ejT_bf_sb")
        hTbf_sb = sp.tile([D, N], BF16, name="hTbf_sb", tag="hTbf_sb")
        eibr_sb = sp.tile([D, R], F32, name="eibr_sb", tag="eibr_sb")
        WhTr_sb = sp.tile([D, R], F32, name="WhTr_sb", tag="WhTr_sb")
        qb_sb = sp.tile([D, R], F32, name="qb_sb", tag="qb_sb")
        u_sb = sp.tile([DK, R], F32, name="u_sb", tag="u_sb")
        Whb_sb = sp.tile([128, 8 * D], BF16, name="Whb_sb", tag="Whb_sb")
        # rhs1: two i-buffers of [128, N]; rows 64:128 hold hT_bf (constant)
        rhs1_sb = sp.tile([128, 3 * N], BF16, name="rhs1_sb", tag="rhs1_sb")
        rhs2_sb = sp.tile([128, 3 * N], BF16, name="rhs2_sb", tag="rhs2_sb")
        e_sb = sp.tile([R, N], F32, name="e_sb", tag="e_sb")
        em_sb = sp.tile([R, N], F32, name="em_sb", tag="em_sb")
        ex_sb = sp.tile([R, N], F32, name="ex_sb", tag="ex_sb")
        attn_sb = sp.tile([R, N], BF16, name="attn_sb", tag="attn_sb")
        attnT_sb = sp.tile([128, N], BF16, name="attnT_sb", tag="attnT_sb")
        scr_sb = sp.tile([1, 8], F32, name="scr_sb", tag="scr_sb")
        red_sb = sp.tile([R, 8], F32, name="red_sb", tag="red_sb")
        hp_sb = sp.tile([R, D], F32, name="hp_sb", tag="hp_sb")
        xm_sb = sp.tile([R, D], F32, name="xm_sb", tag="xm_sb")
        o_sb = sp.tile([R, D], F32, name="o_sb", tag="o_sb")

        # ---------------- load inputs (critical-path order) ----------------
        nc.sync.dma_start(hTbf_sb[:], hT_bf)
        nc.sync.dma_start(Ej_sb[:], Ej)
        nc.sync.dma_start(hTr_sb[:], hTr)
        nc.sync.dma_start(Ei_sb[:], Ei)
        nc.sync.dma_start(eb_sb[:], ebcol)
        nc.sync.dma_start(lhsT1_sb[:], lhsT1)
        nc.sync.dma_start(jlinT_sb[:], jlinT)
        nc.sync.dma_start(hT_sb[:], hT_f)
        nc.sync.dma_start(Wm_sb[:], Wm)
        nc.sync.dma_start(Ai_sb[:], Ai)
        nc.sync.dma_start(Ae_sb[:], Ae)
        nc.sync.dma_start(b1_sb[:], b1col)
        nc.sync.dma_start(lhsT2_sb[:], lhsT2)
        nc.sync.dma_start(jones_sb[:], jones)
        nc.sync.dma_start(rhs1_sb[D:2 * D, 0:N], hT_bf)
        nc.sync.dma_start(rhs1_sb[D:2 * D, N:2 * N], hT_bf)
        nc.sync.dma_start(rhs1_sb[D:2 * D, 2 * N:3 * N], hT_bf)
        nc.sync.dma_start(hrows_sb[:], hrows)
        nc.sync.dma_start(iden_sb[:], iden)
        nc.sync.dma_start(lngr_sb[:], lngr)
        nc.sync.dma_start(lnbr_sb[:], lnbr)
        nc.sync.dma_start(adjm_sb[:], adjm)

        # zero the stage-3 rhs pad rows once (zero weights x junk = NaN risk)
        nc.vector.memset(rhs2_sb[D:128, :], 0.0)

        # warm ACT table sets early (exp/ln)
        nc.vector.memset(scr_sb[:], 1.0)
        nc.scalar.activation(scr_sb[0:1, 0:1], scr_sb[0:1, 1:2], AF.Exp)
        nc.scalar.activation(scr_sb[0:1, 2:3], scr_sb[0:1, 3:4], AF.Ln)

        # ---------------- setup math ----------------
        with tc.tile_pool(name="ps_setup", bufs=1, space="PSUM") as psp:
            # ejT (bf16) over all N columns, from the bf16 h copy
            for jh in range(2):
                ej_ps = psp.tile([D, 512], F32, name="ej_ps", bufs=2)
                nc.tensor.matmul(ej_ps[:], Ej_sb[:],
                                 hTbf_sb[:, jh * 512:(jh + 1) * 512])
                nc.vector.tensor_copy(ejT_bf_sb[:, jh * 512:(jh + 1) * 512], ej_ps[:])
            # PE pre-warm: 12 loop-shaped matmuls on loaded data (results unused)
            # so HAM reaches 2.4 GHz before the main loop starts
            wd_ps = psp.tile([DK, 512], F32, name="wd_ps")
            for _ in range(12):
                nc.tensor.matmul(wd_ps[:], lhsT1_sb[:], jlinT_sb[:, 0:512])
            # WhTr = W^T-projected rows (feature-major, this core's columns)
            whtr_ps = psp.tile([D, R], F32, name="whtr_ps", tag="small_ps", bufs=2)
            nc.tensor.matmul(whtr_ps[:], Wm_sb[:], hTr_sb[:])
            nc.vector.tensor_copy(WhTr_sb[:], whtr_ps[:])
            # eibr = E_i^T h_rows + edge_b
            eib_ps = psp.tile([D, R], F32, name="eib_ps", tag="small_ps", bufs=2)
            nc.tensor.matmul(eib_ps[:], Ei_sb[:], hTr_sb[:])
            nc.vector.tensor_scalar(eibr_sb[:], eib_ps[:], eb_sb[:], None, op0=ALU.add)
            # qb = (A_i diag(c))^T WhTr + c*b1
            q_ps = psp.tile([D, R], F32, name="q_ps", tag="small_ps", bufs=2)
            nc.tensor.matmul(q_ps[:], Ai_sb[:], WhTr_sb[:])
            nc.vector.tensor_scalar(qb_sb[:], q_ps[:], b1_sb[:], None, op0=ALU.add)
            # u[0:64] = qb + ALPHA * (A_e diag(c))^T eibr ; rows 64,65 = 0
            z_ps = psp.tile([D, R], F32, name="z_ps", tag="small_ps", bufs=2)
            nc.tensor.matmul(z_ps[:], Ae_sb[:], eibr_sb[:])
            nc.vector.scalar_tensor_tensor(
                u_sb[0:D, :], z_ps[:], ALPHA, qb_sb[:], op0=ALU.mult, op1=ALU.add)
            nc.vector.memset(u_sb[D:DK, :], 0.0)
            # Wh node-major [128, 64] x 8 tiles (bf16 for the final matmul)
            for t in range(8):
                wh_ps = psp.tile([128, D], F32, name="wh_ps", bufs=2)
                nc.tensor.matmul(wh_ps[:], hT_sb[:, t * 128:(t + 1) * 128], Wm_sb[:])
                nc.vector.tensor_copy(Whb_sb[:, t * D:(t + 1) * D], wh_ps[:])

        # ---------------- main loop over this core's 128 rows ----------------
        def fill(j):
            # rhs1 upper half for row j: relu(ei_j + ejT + eb)
            fb = j % 3
            nc.vector.tensor_scalar(
                rhs1_sb[0:D, fb * N:(fb + 1) * N],
                ejT_bf_sb[:],
                eibr_sb[:, j:j + 1], 0.0, op0=ALU.add, op1=ALU.max)

        with tc.tile_pool(name="ps_mm1", bufs=3, space="PSUM") as pmm1, \
             tc.tile_pool(name="ps_e", bufs=2, space="PSUM") as pe:
            fill(0)
            fill(1)
            bankE = None
            for i in range(R):
                g = i % 32
                grp = i // 32
                buf = i % 3
                if g == 0:
                    bankE = [pe.tile([DK, 512], F32, name="bankE", tag="bankE")
                             for _ in range(2)]
                if i + 2 < R:
                    fill(i + 2)
                # main matmul: psum1[0:64] = c*(t+v) ; rows 64,65 = +-slin
                psum1 = pmm1.tile([DK, N], F32, name="psum1", tag="psum1")
                for jh in range(2):
                    nc.tensor.matmul(
                        psum1[:, jh * 512:(jh + 1) * 512],
                        lhsT1_sb[:],
                        rhs1_sb[:, buf * N + jh * 512: buf * N + (jh + 1) * 512])
                # stage 2: rhs2 = relu(psum1 + u)  (bf16)
                r2 = rhs2_sb[0:DK, buf * N:(buf + 1) * N]
                if i % 4 == 0:
                    nc.vector.tensor_scalar(
                        r2, psum1[:], u_sb[:, i:i + 1], 0.0,
                        op0=ALU.add, op1=ALU.max)
                else:
                    nc.scalar.activation(r2, psum1[:], AF.Relu,
                                         bias=u_sb[:, i:i + 1], scale=1.0)
                # score matmul: accumulate e rows into persistent banks
                for jh in range(2):
                    nc.tensor.matmul(
                        bankE[jh][:],
                        lhsT2_sb[:, g * 128:g * 128 + DK],
                        rhs2_sb[:, buf * N + jh * 512: buf * N + (jh + 1) * 512],
                        start=(g == 0), stop=False)
                if g == 31:
                    # add per-j linear part (jlin) to all 32 rows, close group
                    for jh in range(2):
                        nc.tensor.matmul(
                            bankE[jh][:],
                            jones_sb[:],
                            jlinT_sb[:, jh * 512:(jh + 1) * 512],
                            start=False, stop=True)
                    for jh in range(2):
                        dst = e_sb[grp * 32:(grp + 1) * 32,
                                   jh * 512:(jh + 1) * 512]
                        if (grp + jh) % 2 == 0:
                            nc.vector.tensor_copy(dst, bankE[jh][0:32, :])
                        else:
                            nc.scalar.copy(dst, bankE[jh][0:32, :])

        # ---------------- mask + softmax ----------------
        nc.vector.tensor_tensor(em_sb[:], e_sb[:], adjm_sb[:], op=ALU.add)
        nc.vector.reduce_max(red_sb[:, 0:1], em_sb[:], axis=AX.X)
        nc.vector.tensor_scalar(red_sb[:, 1:2], red_sb[:, 0:1], -1.0, None,
                                op0=ALU.mult)
        nc.scalar.activation(ex_sb[:], em_sb[:], AF.Exp,
                             bias=red_sb[:, 1:2], scale=1.0,
                             accum_out=red_sb[:, 2:3])
        nc.vector.reciprocal(red_sb[:, 3:4], red_sb[:, 2:3])
        nc.vector.tensor_scalar(attn_sb[:], ex_sb[:], red_sb[:, 3:4], None,
                                op0=ALU.mult)

        # ---------------- h' = attn @ Wh + h ; LayerNorm ----------------
        with tc.tile_pool(name="ps_fin", bufs=4, space="PSUM") as pf:
            for t in range(8):
                tp_ps = pf.tile([128, 128], BF16, name="tp_ps", tag="tp")
                nc.tensor.transpose(tp_ps[:], attn_sb[:, t * 128:(t + 1) * 128],
                                    iden_sb[:])
                nc.vector.tensor_copy(attnT_sb[:, t * 128:(t + 1) * 128], tp_ps[:])
            hp_ps = pf.tile([R, D], F32, name="hp_ps", bufs=1)
            for t in range(8):
                nc.tensor.matmul(hp_ps[:], attnT_sb[:, t * 128:(t + 1) * 128],
                                 Whb_sb[:, t * D:(t + 1) * D],
                                 start=(t == 0), stop=(t == 7))
            nc.vector.tensor_tensor(hp_sb[:], hp_ps[:], hrows_sb[:], op=ALU.add)

        nc.vector.reduce_sum(red_sb[:, 4:5], hp_sb[:], axis=AX.X)
        nc.vector.tensor_scalar(red_sb[:, 5:6], red_sb[:, 4:5], 1.0 / D, None,
                                op0=ALU.mult)
        nc.vector.tensor_scalar(xm_sb[:], hp_sb[:], red_sb[:, 5:6], None,
                                op0=ALU.subtract)
        nc.vector.tensor_tensor(o_sb[:], xm_sb[:], xm_sb[:], op=ALU.mult)
        nc.vector.reduce_sum(red_sb[:, 6:7], o_sb[:], axis=AX.X)
        # rstd = exp(-0.5 * ln(var + eps))
        nc.vector.tensor_scalar(red_sb[:, 6:7], red_sb[:, 6:7], 1.0 / D,
                                LN_EPS, op0=ALU.mult, op1=ALU.add)
        nc.scalar.activation(red_sb[:, 7:8], red_sb[:, 6:7], AF.Ln)
        nc.scalar.activation(red_sb[:, 7:8], red_sb[:, 7:8], AF.Exp,
                             bias=0.0, scale=-0.5)
        nc.vector.tensor_scalar(xm_sb[:], xm_sb[:], red_sb[:, 7:8], None,
                                op0=ALU.mult)
        nc.vector.tensor_tensor(o_sb[:], xm_sb[:], lngr_sb[:], op=ALU.mult)
        nc.vector.tensor_tensor(o_sb[:], o_sb[:], lnbr_sb[:], op=ALU.add)
        nc.sync.dma_start(out_d, o_sb[:])

    nc.compile()
    return nc


def _host_prep(inputs):
    h = np.asarray(inputs["h"], np.float32)[0]            # [N, D]
    adj = np.asarray(inputs["adj"])[0]                    # [N, N] int32
    W = np.asarray(inputs["W"], np.float32)
    attn_w1 = np.asarray(inputs["attn_w1"], np.float32)
    attn_b1 = np.asarray(inputs["attn_b1"], np.float32)
    attn_w2 = np.asarray(inputs["attn_w2"], np.float32)
    edge_w = np.asarray(inputs["edge_w"], np.float32)
    edge_b = np.asarray(inputs["edge_b"], np.float32)
    ln_g = np.asarray(inputs["ln_g"], np.float32)
    ln_b = np.asarray(inputs["ln_b"], np.float32)

    A_i, A_j, A_e = attn_w1[:D], attn_w1[D:2 * D], attn_w1[2 * D:]
    E_i, E_j = edge_w[:D], edge_w[D:]
    w2 = attn_w2[:, 0]

    hT = np.ascontiguousarray(h.T)                        # [D, N]
    Mv = W @ A_j + ALPHA * (E_j @ A_e)
    c = 0.8 * np.abs(w2)
    sgn = np.sign(w2).astype(np.float32)

    lhsT1 = np.zeros((2 * D, DK), np.float32)
    lhsT1[:D, :D] = 0.8 * A_e * c[None, :]
    lhsT1[D:, :D] = Mv * c[None, :]
    slw = 0.8 * ALPHA * (A_e @ w2)
    lhsT1[:D, D] = slw
    lhsT1[:D, D + 1] = -slw

    lhsT2 = np.zeros((128, 32 * 128), np.float32)
    for g in range(32):
        lhsT2[:D, g * 128 + g] = sgn
        lhsT2[D, g * 128 + g] = 1.0
        lhsT2[D + 1, g * 128 + g] = -1.0

    jones = np.zeros((128, DK), np.float32)
    jones[0, 0:32] = 1.0

    jlin = ALPHA * (h @ (Mv @ w2))                        # [N]
    jlinT = np.zeros((128, N), np.float32)
    jlinT[0] = jlin

    rep = {
        "hT_f": hT,
        "hT_bf": hT.astype(ml_dtypes.bfloat16),
        "lhsT1": lhsT1.astype(ml_dtypes.bfloat16),
        "lhsT2": lhsT2.astype(ml_dtypes.bfloat16),
        "jones": jones.astype(ml_dtypes.bfloat16),
        "jlinT": jlinT.astype(ml_dtypes.bfloat16),
        "Ej": np.ascontiguousarray(E_j).astype(ml_dtypes.bfloat16),
        "Ei": np.ascontiguousarray(E_i),
        "Wm": W,
        "Ai": np.ascontiguousarray(A_i * c[None, :]),
        "Ae": np.ascontiguousarray(A_e * c[None, :]),
        "b1col": np.ascontiguousarray((c * attn_b1)[:, None]),
        "ebcol": np.ascontiguousarray(edge_b[:, None]),
        "iden": np.eye(128, dtype=ml_dtypes.bfloat16),
        "lngr": np.broadcast_to(ln_g, (R, D)).copy(),
        "lnbr": np.broadcast_to(ln_b, (R, D)).copy(),
    }
    in_maps = []
    for cid in range(NCORES):
        rows = slice(cid * R, (cid + 1) * R)
        m = dict(rep)
        m["hTr"] = np.ascontiguousarray(hT[:, rows])
        m["hrows"] = np.ascontiguousarray(h[rows])
        m["adjm"] = np.where(adj[rows] == 0, np.float32(-30000.0),
                             np.float32(0.0))
        in_maps.append(m)
    return in_maps


def _get_nc():
    if "nc" not in _CACHE:
        _CACHE["nc"] = _build_program()
    return _CACHE["nc"]


def kernel(**inputs) -> np.ndarray:
    nc = _get_nc()
    in_maps = _host_prep(inputs)
    res = run_bass_kernel_spmd(nc, in_maps, list(range(NCORES))).results
    out = np.concatenate([res[c]["out"] for c in range(NCORES)], axis=0)
    return out[None].astype(np.float32)


# revision 15
# speedup vs baseline: 3.9808x; 1.0361x over previous
"""Trainium2 Bass kernel for EnhancedGraphAttentionLayer (B=1, N=1024, D=64).

Sharding: destination-node rows split across 8 cores (128 rows each).
Each core is fully independent (no collectives): it holds h replicated and
computes its 128 rows of scores/softmax/attention locally.

v2 decomposition (all-bf16 matmuls):
  LeakyReLU(x) = 0.2*x + 0.8*relu(x) at both nonlinearities.
  pre_ij = u_i + 0.8*A_e^T relu(s_ij) + Mv^T h_j,  s_ij = ei_i + ej_j + eb
  e_ij  = 0.8*sum_k w2_k relu(pre_k) + 0.2*w2^T(t+v) + const_i
  Fold c_k = 0.8*|w2_k| into stage-1 weight cols + u rows (LeakyReLU is
  positively homogeneous), so stage-3 reduces with exact +-1 signs in bf16.
  The 0.2-linear score part splits into:
    slin (from relu(s)): carried as two extra +-slin psum rows
      (relu(x)-relu(-x)=x, exact), reduced with +1/-1 in stage-3;
    jlin (per-j const): host-precomputed row vector, added into the score
      psum bank with one ones-weight matmul pair per 32-row group.
  Mask is additive (-30000 on adj==0), so no relu-safe score shift needed.
"""
import sys
import numpy as np

if "/opt/trn_rl_repo" not in sys.path:
    sys.path.insert(0, "/opt/trn_rl_repo")

import ml_dtypes
import concourse.bass as bass
import concourse.bacc as bacc
import concourse.mybir as mybir
import concourse.tile as tile
from concourse.bass_utils import run_bass_kernel_spmd

F32 = mybir.dt.float32
BF16 = mybir.dt.bfloat16
AF = mybir.ActivationFunctionType
ALU = mybir.AluOpType
AX = mybir.AxisListType

N = 1024
D = 64
NCORES = 8
R = N // NCORES          # 128 rows per core
ALPHA = 0.2
LN_EPS = 1e-5
DK = D + 2               # 64 features + slin+/- rows

_CACHE = {}


def _build_program():
    nc = bacc.Bacc("TRN2", target_bir_lowering=False, debug=False,
                   num_devices=NCORES)

    def din(name, shape, dt):
        return nc.dram_tensor(name, shape, dt, kind="ExternalInput").ap()

    hT_f = din("hT_f", [D, N], F32)
    hT_bf = din("hT_bf", [D, N], BF16)
    hTr = din("hTr", [D, R], F32)
    hrows = din("hrows", [R, D], F32)
    adjm = din("adjm", [R, N], F32)
    lhsT1 = din("lhsT1", [2 * D, DK], BF16)
    # stage-3 weights padded to the same [128, 66] shape as lhsT1 so the PE
    # never switches weight tile shape (shape alternation blocks HAM warm-up)
    lhsT2 = din("lhsT2", [128, 32 * 128], BF16)
    jones = din("jones", [128, DK], BF16)
    jlinT = din("jlinT", [128, N], BF16)
    Ej = din("Ej", [D, D], BF16)
    # packed fp32 params: Ei | Wm | Ai(c-scaled) | Ae(c-scaled) | c*b1 | edge_b
    park = din("park", [D, 4 * D + 2], F32)
    iden = din("iden", [128, 128], BF16)
    lngr = din("lngr", [R, D], F32)
    lnbr = din("lnbr", [R, D], F32)
    out_d = nc.dram_tensor("out", [R, D], F32, kind="ExternalOutput").ap()

    with tile.TileContext(nc) as tc, \
         tc.tile_pool(name="static", bufs=1) as sp:
        # ---------------- static SBUF tiles ----------------
        hT_sb = sp.tile([D, N], F32, name="hT_sb", tag="hT_sb")
        hTr_sb = sp.tile([D, R], F32, name="hTr_sb", tag="hTr_sb")
        hrows_sb = sp.tile([R, D], F32, name="hrows_sb", tag="hrows_sb")
        adjm_sb = sp.tile([R, N], F32, name="adjm_sb", tag="adjm_sb")
        lhsT1_sb = sp.tile([2 * D, DK], BF16, name="lhsT1_sb", tag="lhsT1_sb")
        lhsT2_sb = sp.tile([128, 32 * 128], BF16, name="lhsT2_sb", tag="lhsT2_sb")
        jones_sb = sp.tile([128, DK], BF16, name="jones_sb", tag="jones_sb")
        jlinT_sb = sp.tile([128, N], BF16, name="jlinT_sb", tag="jlinT_sb")
        Ej_sb = sp.tile([D, D], BF16, name="Ej_sb", tag="Ej_sb")
        park_sb = sp.tile([D, 4 * D + 2], F32, name="park_sb", tag="park_sb")
        Ei_sb = park_sb[:, 0:D]
        Wm_sb = park_sb[:, D:2 * D]
        Ai_sb = park_sb[:, 2 * D:3 * D]
        Ae_sb = park_sb[:, 3 * D:4 * D]
        b1_sb = park_sb[:, 4 * D:4 * D + 1]
        eb_sb = park_sb[:, 4 * D + 1:4 * D + 2]
        iden_sb = sp.tile([128, 128], BF16, name="iden_sb", tag="iden_sb")
        lngr_sb = sp.tile([R, D], F32, name="lngr_sb", tag="lngr_sb")
        lnbr_sb = sp.tile([R, D], F32, name="lnbr_sb", tag="lnbr_sb")

        ejT_bf_sb = sp.tile([D, N], BF16, name="ejT_bf_sb", tag="ejT_bf_sb")
        hTbf_sb = sp.tile([D, N], BF16, name="hTbf_sb", tag="hTbf_sb")
        eibr_sb = sp.tile([D, R], F32, name="eibr_sb", tag="eibr_sb")
        WhTr_sb = sp.tile([D, R], F32, name="WhTr_sb", tag="WhTr_sb")
        qb_sb = sp.tile([D, R], F32, name="qb_sb", tag="qb_sb")
        u_sb = sp.tile([DK, R], F32, name="u_sb", tag="u_sb")
        Whb_sb = sp.tile([128, 8 * D], BF16, name="Whb_sb", tag="Whb_sb")
        # rhs1: two i-buffers of [128, N]; rows 64:128 hold hT_bf (constant)
        rhs1_sb = sp.tile([128, 3 * N], BF16, name="rhs1_sb", tag="rhs1_sb")
        rhs2_sb = sp.tile([128, 3 * N], BF16, name="rhs2_sb", tag="rhs2_sb")
        e_sb = sp.tile([R, N], F32, name="e_sb", tag="e_sb")
        em_sb = sp.tile([R, N], F32, name="em_sb", tag="em_sb")
        ex_sb = sp.tile([R, N], F32, name="ex_sb", tag="ex_sb")
        attn_sb = sp.tile([R, N], BF16, name="attn_sb", tag="attn_sb")
        attnT_sb = sp.tile([128, N], BF16, name="attnT_sb", tag="attnT_sb")
        scr_sb = sp.tile([1, 8], F32, name="scr_sb", tag="scr_sb")
        red_sb = sp.tile([R, 8], F32, name="red_sb", tag="red_sb")
        hp_sb = sp.tile([R, D], F32, name="hp_sb", tag="hp_sb")
        xm_sb = sp.tile([R, D], F32, name="xm_sb", tag="xm_sb")
        o_sb = sp.tile([R, D], F32, name="o_sb", tag="o_sb")

        # ---------------- load inputs (critical-path order) ----------------
        nc.sync.dma_start(hTbf_sb[:], hT_bf)
        nc.sync.dma_start(Ej_sb[:], Ej)
        nc.sync.dma_start(hTr_sb[:], hTr)
        nc.sync.dma_start(park_sb[:], park)
        nc.sync.dma_start(lhsT1_sb[:], lhsT1)
        nc.sync.dma_start(jlinT_sb[:], jlinT)
        nc.sync.dma_start(hT_sb[:], hT_f)
        nc.sync.dma_start(lhsT2_sb[:], lhsT2)
        nc.sync.dma_start(jones_sb[:], jones)
        nc.sync.dma_start(rhs1_sb[D:2 * D, 0:N], hT_bf)
        nc.sync.dma_start(rhs1_sb[D:2 * D, N:2 * N], hT_bf)
        nc.sync.dma_start(rhs1_sb[D:2 * D, 2 * N:3 * N], hT_bf)
        nc.sync.dma_start(hrows_sb[:], hrows)
        nc.sync.dma_start(iden_sb[:], iden)
        nc.sync.dma_start(lngr_sb[:], lngr)
        nc.sync.dma_start(lnbr_sb[:], lnbr)
        nc.sync.dma_start(adjm_sb[:], adjm)

        # zero the stage-3 rhs pad rows once (zero weights x junk = NaN risk)
        nc.vector.memset(rhs2_sb[D:128, :], 0.0)

        # warm the ACT exp table (exp/relu/copy share one set)
        nc.vector.memset(scr_sb[:], 1.0)
        nc.scalar.activation(scr_sb[0:1, 0:1], scr_sb[0:1, 1:2], AF.Exp)

        # ---------------- setup math ----------------
        with tc.tile_pool(name="ps_setup", bufs=1, space="PSUM") as psp:
            # ejT (bf16) over all N columns, from the bf16 h copy
            for jh in range(2):
                ej_ps = psp.tile([D, 512], F32, name="ej_ps", bufs=2)
                nc.tensor.matmul(ej_ps[:], Ej_sb[:],
                                 hTbf_sb[:, jh * 512:(jh + 1) * 512])
                nc.vector.tensor_copy(ejT_bf_sb[:, jh * 512:(jh + 1) * 512], ej_ps[:])
            # PE pre-warm: 12 loop-shaped matmuls on loaded data (results unused)
            # so HAM reaches 2.4 GHz before the main loop starts
            wd_ps = psp.tile([DK, 512], F32, name="wd_ps")
            for _ in range(12):
                nc.tensor.matmul(wd_ps[:], lhsT1_sb[:], jlinT_sb[:, 0:512])
            # WhTr = W^T-projected rows (feature-major, this core's columns)
            whtr_ps = psp.tile([D, R], F32, name="whtr_ps", tag="small_ps", bufs=2)
            nc.tensor.matmul(whtr_ps[:], Wm_sb, hTr_sb[:])
            nc.vector.tensor_copy(WhTr_sb[:], whtr_ps[:])
            # eibr = E_i^T h_rows + edge_b
            eib_ps = psp.tile([D, R], F32, name="eib_ps", tag="small_ps", bufs=2)
            nc.tensor.matmul(eib_ps[:], Ei_sb, hTr_sb[:])
            nc.vector.tensor_scalar(eibr_sb[:], eib_ps[:], eb_sb, None, op0=ALU.add)
            # qb = (A_i diag(c))^T WhTr + c*b1
            q_ps = psp.tile([D, R], F32, name="q_ps", tag="small_ps", bufs=2)
            nc.tensor.matmul(q_ps[:], Ai_sb, WhTr_sb[:])
            nc.vector.tensor_scalar(qb_sb[:], q_ps[:], b1_sb, None, op0=ALU.add)
            # u[0:64] = qb + ALPHA * (A_e diag(c))^T eibr ; rows 64,65 = 0
            z_ps = psp.tile([D, R], F32, name="z_ps", tag="small_ps", bufs=2)
            nc.tensor.matmul(z_ps[:], Ae_sb, eibr_sb[:])
            nc.vector.scalar_tensor_tensor(
                u_sb[0:D, :], z_ps[:], ALPHA, qb_sb[:], op0=ALU.mult, op1=ALU.add)
            nc.vector.memset(u_sb[D:DK, :], 0.0)
            # Wh node-major [128, 64] x 8 tiles (bf16 for the final matmul)
            for t in range(8):
                wh_ps = psp.tile([128, D], F32, name="wh_ps", bufs=2)
                nc.tensor.matmul(wh_ps[:], hT_sb[:, t * 128:(t + 1) * 128], Wm_sb)
                nc.vector.tensor_copy(Whb_sb[:, t * D:(t + 1) * D], wh_ps[:])

        # ---------------- main loop over this core's 128 rows ----------------
        def fill(j):
            # rhs1 upper half for row j: relu(ei_j + ejT + eb)
            fb = j % 3
            nc.vector.tensor_scalar(
                rhs1_sb[0:D, fb * N:(fb + 1) * N],
                ejT_bf_sb[:],
                eibr_sb[:, j:j + 1], 0.0, op0=ALU.add, op1=ALU.max)

        with tc.tile_pool(name="ps_mm1", bufs=3, space="PSUM") as pmm1, \
             tc.tile_pool(name="ps_e", bufs=2, space="PSUM") as pe:
            fill(0)
            fill(1)
            bankE = None
            for i in range(R):
                g = i % 32
                grp = i // 32
                buf = i % 3
                if g == 0:
                    bankE = [pe.tile([DK, 512], F32, name="bankE", tag="bankE")
                             for _ in range(2)]
                if i + 2 < R:
                    fill(i + 2)
                # main matmul: psum1[0:64] = c*(t+v) ; rows 64,65 = +-slin
                psum1 = pmm1.tile([DK, N], F32, name="psum1", tag="psum1")
                for jh in range(2):
                    nc.tensor.matmul(
                        psum1[:, jh * 512:(jh + 1) * 512],
                        lhsT1_sb[:],
                        rhs1_sb[:, buf * N + jh * 512: buf * N + (jh + 1) * 512])
                # stage 2: rhs2 = relu(psum1 + u)  (bf16)
                r2 = rhs2_sb[0:DK, buf * N:(buf + 1) * N]
                if i % 4 == 0:
                    nc.vector.tensor_scalar(
                        r2, psum1[:], u_sb[:, i:i + 1], 0.0,
                        op0=ALU.add, op1=ALU.max)
                else:
                    nc.scalar.activation(r2, psum1[:], AF.Relu,
                                         bias=u_sb[:, i:i + 1], scale=1.0)
                # score matmul: accumulate e rows into persistent banks
                for jh in range(2):
                    nc.tensor.matmul(
                        bankE[jh][:],
                        lhsT2_sb[:, g * 128:g * 128 + DK],
                        rhs2_sb[:, buf * N + jh * 512: buf * N + (jh + 1) * 512],
                        start=(g == 0), stop=False)
                if g == 31:
                    # add per-j linear part (jlin) to all 32 rows, close group
                    for jh in range(2):
                        nc.tensor.matmul(
                            bankE[jh][:],
                            jones_sb[:],
                            jlinT_sb[:, jh * 512:(jh + 1) * 512],
                            start=False, stop=True)
                    for jh in range(2):
                        dst = e_sb[grp * 32:(grp + 1) * 32,
                                   jh * 512:(jh + 1) * 512]
                        if (grp + jh) % 2 == 0:
                            nc.vector.tensor_copy(dst, bankE[jh][0:32, :])
                        else:
                            nc.scalar.copy(dst, bankE[jh][0:32, :])

        # ---------------- mask + softmax ----------------
        nc.vector.tensor_tensor(em_sb[:], e_sb[:], adjm_sb[:], op=ALU.add)
        nc.vector.reduce_max(red_sb[:, 0:1], em_sb[:], axis=AX.X)
        nc.vector.tensor_scalar(red_sb[:, 1:2], red_sb[:, 0:1], -1.0, None,
                                op0=ALU.mult)
        nc.scalar.activation(ex_sb[:], em_sb[:], AF.Exp,
                             bias=red_sb[:, 1:2], scale=1.0,
                             accum_out=red_sb[:, 2:3])
        nc.vector.reciprocal(red_sb[:, 3:4], red_sb[:, 2:3])
        nc.vector.tensor_scalar(attn_sb[:], ex_sb[:], red_sb[:, 3:4], None,
                                op0=ALU.mult)

        # ---------------- h' = attn @ Wh + h ; LayerNorm ----------------
        with tc.tile_pool(name="ps_fin", bufs=4, space="PSUM") as pf:
            for t in range(8):
                tp_ps = pf.tile([128, 128], BF16, name="tp_ps", tag="tp")
                nc.tensor.transpose(tp_ps[:], attn_sb[:, t * 128:(t + 1) * 128],
                                    iden_sb[:])
                nc.vector.tensor_copy(attnT_sb[:, t * 128:(t + 1) * 128], tp_ps[:])
            hp_ps = pf.tile([R, D], F32, name="hp_ps", bufs=1)
            for t in range(8):
                nc.tensor.matmul(hp_ps[:], attnT_sb[:, t * 128:(t + 1) * 128],
                                 Whb_sb[:, t * D:(t + 1) * D],
                                 start=(t == 0), stop=(t == 7))
            nc.vector.tensor_tensor(hp_sb[:], hp_ps[:], hrows_sb[:], op=ALU.add)

        nc.vector.reduce_sum(red_sb[:, 4:5], hp_sb[:], axis=AX.X)
        nc.vector.tensor_scalar(red_sb[:, 5:6], red_sb[:, 4:5], 1.0 / D, None,
                                op0=ALU.mult)
        nc.vector.tensor_scalar(xm_sb[:], hp_sb[:], red_sb[:, 5:6], None,
                                op0=ALU.subtract)
        nc.vector.tensor_tensor(o_sb[:], xm_sb[:], xm_sb[:], op=ALU.mult)
        nc.vector.reduce_sum(red_sb[:, 6:7], o_sb[:], axis=AX.X)
        # rstd = 1 / sqrt(var + eps)
        nc.vector.tensor_scalar(red_sb[:, 6:7], red_sb[:, 6:7], 1.0 / D,
                                LN_EPS, op0=ALU.mult, op1=ALU.add)
        nc.scalar.activation(red_sb[:, 7:8], red_sb[:, 6:7], AF.Sqrt)
        nc.vector.reciprocal(red_sb[:, 7:8], red_sb[:, 7:8])
        nc.vector.tensor_scalar(xm_sb[:], xm_sb[:], red_sb[:, 7:8], None,
                                op0=ALU.mult)
        nc.vector.tensor_tensor(o_sb[:], xm_sb[:], lngr_sb[:], op=ALU.mult)
        nc.vector.tensor_tensor(o_sb[:], o_sb[:], lnbr_sb[:], op=ALU.add)
        nc.sync.dma_start(out_d, o_sb[:])

    nc.compile()
    return nc


def _host_prep(inputs):
    h = np.asarray(inputs["h"], np.float32)[0]            # [N, D]
    adj = np.asarray(inputs["adj"])[0]                    # [N, N] int32
    W = np.asarray(inputs["W"], np.float32)
    attn_w1 = np.asarray(inputs["attn_w1"], np.float32)
    attn_b1 = np.asarray(inputs["attn_b1"], np.float32)
    attn_w2 = np.asarray(inputs["attn_w2"], np.float32)
    edge_w = np.asarray(inputs["edge_w"], np.float32)
    edge_b = np.asarray(inputs["edge_b"], np.float32)
    ln_g = np.asarray(inputs["ln_g"], np.float32)
    ln_b = np.asarray(inputs["ln_b"], np.float32)

    A_i, A_j, A_e = attn_w1[:D], attn_w1[D:2 * D], attn_w1[2 * D:]
    E_i, E_j = edge_w[:D], edge_w[D:]
    w2 = attn_w2[:, 0]

    hT = np.ascontiguousarray(h.T)                        # [D, N]
    Mv = W @ A_j + ALPHA * (E_j @ A_e)
    c = 0.8 * np.abs(w2)
    sgn = np.sign(w2).astype(np.float32)

    lhsT1 = np.zeros((2 * D, DK), np.float32)
    lhsT1[:D, :D] = 0.8 * A_e * c[None, :]
    lhsT1[D:, :D] = Mv * c[None, :]
    slw = 0.8 * ALPHA * (A_e @ w2)
    lhsT1[:D, D] = slw
    lhsT1[:D, D + 1] = -slw

    lhsT2 = np.zeros((128, 32 * 128), np.float32)
    for g in range(32):
        lhsT2[:D, g * 128 + g] = sgn
        lhsT2[D, g * 128 + g] = 1.0
        lhsT2[D + 1, g * 128 + g] = -1.0

    jones = np.zeros((128, DK), np.float32)
    jones[0, 0:32] = 1.0

    jlin = ALPHA * (h @ (Mv @ w2))                        # [N]
    jlinT = np.zeros((128, N), np.float32)
    jlinT[0] = jlin

    rep = {
        "hT_f": hT,
        "hT_bf": hT.astype(ml_dtypes.bfloat16),
        "lhsT1": lhsT1.astype(ml_dtypes.bfloat16),
        "lhsT2": lhsT2.astype(ml_dtypes.bfloat16),
        "jones": jones.astype(ml_dtypes.bfloat16),
        "jlinT": jlinT.astype(ml_dtypes.bfloat16),
        "Ej": np.ascontiguousarray(E_j).astype(ml_dtypes.bfloat16),
        "park": np.concatenate(
            [E_i, W, A_i * c[None, :], A_e * c[None, :],
             (c * attn_b1)[:, None], edge_b[:, None]], axis=1).astype(np.float32),
        "iden": np.eye(128, dtype=ml_dtypes.bfloat16),
        "lngr": np.broadcast_to(ln_g, (R, D)).copy(),
        "lnbr": np.broadcast_to(ln_b, (R, D)).copy(),
    }
    in_maps = []
    for cid in range(NCORES):
        rows = slice(cid * R, (cid + 1) * R)
        m = dict(rep)
        m["hTr"] = np.ascontiguousarray(hT[:, rows])
        m["hrows"] = np.ascontiguousarray(h[rows])
        m["adjm"] = np.where(adj[rows] == 0, np.float32(-30000.0),
                             np.float32(0.0))
        in_maps.append(m)
    return in_maps


def _get_nc():
    if "nc" not in _CACHE:
        _CACHE["nc"] = _build_program()
    return _CACHE["nc"]


def kernel(**inputs) -> np.ndarray:
    nc = _get_nc()
    in_maps = _host_prep(inputs)
    res = run_bass_kernel_spmd(nc, in_maps, list(range(NCORES))).results
    out = np.concatenate([res[c]["out"] for c in range(NCORES)], axis=0)
    return out[None].astype(np.float32)


# revision 17
# speedup vs baseline: 4.0668x; 1.0216x over previous
"""Trainium2 Bass kernel for EnhancedGraphAttentionLayer (B=1, N=1024, D=64).

Sharding: destination-node rows split across 8 cores (128 rows each).
Each core is fully independent (no collectives): it holds h replicated and
computes its 128 rows of scores/softmax/attention locally.

v2 decomposition (all-bf16 matmuls):
  LeakyReLU(x) = 0.2*x + 0.8*relu(x) at both nonlinearities.
  pre_ij = u_i + 0.8*A_e^T relu(s_ij) + Mv^T h_j,  s_ij = ei_i + ej_j + eb
  e_ij  = 0.8*sum_k w2_k relu(pre_k) + 0.2*w2^T(t+v) + const_i
  Fold c_k = 0.8*|w2_k| into stage-1 weight cols + u rows (LeakyReLU is
  positively homogeneous), so stage-3 reduces with exact +-1 signs in bf16.
  The 0.2-linear score part splits into:
    slin (from relu(s)): carried as two extra +-slin psum rows
      (relu(x)-relu(-x)=x, exact), reduced with +1/-1 in stage-3;
    jlin (per-j const): host-precomputed row vector, added into the score
      psum bank with one ones-weight matmul pair per 32-row group.
  Mask is additive (-30000 on adj==0), so no relu-safe score shift needed.
"""
import sys
import numpy as np

if "/opt/trn_rl_repo" not in sys.path:
    sys.path.insert(0, "/opt/trn_rl_repo")

import ml_dtypes
import concourse.bass as bass
import concourse.bacc as bacc
import concourse.mybir as mybir
import concourse.tile as tile
from concourse.bass_utils import run_bass_kernel_spmd

F32 = mybir.dt.float32
BF16 = mybir.dt.bfloat16
AF = mybir.ActivationFunctionType
ALU = mybir.AluOpType
AX = mybir.AxisListType

N = 1024
D = 64
NCORES = 8
R = N // NCORES          # 128 rows per core
ALPHA = 0.2
LN_EPS = 1e-5
DK = D + 2               # 64 features + slin+/- rows

_CACHE = {}


def _build_program():
    nc = bacc.Bacc("TRN2", target_bir_lowering=False, debug=False,
                   num_devices=NCORES)

    def din(name, shape, dt):
        return nc.dram_tensor(name, shape, dt, kind="ExternalInput").ap()

    hT_f = din("hT_f", [D, N], F32)
    hT_bf = din("hT_bf", [D, N], BF16)
    hTr = din("hTr", [D, R], F32)
    hrows = din("hrows", [R, D], F32)
    adjm = din("adjm", [R, N], F32)
    lhsT1 = din("lhsT1", [2 * D, DK], BF16)
    # stage-3 weights padded to the same [128, 66] shape as lhsT1 so the PE
    # never switches weight tile shape (shape alternation blocks HAM warm-up)
    lhsT2 = din("lhsT2", [128, 32 * 128], BF16)
    jones = din("jones", [128, DK], BF16)
    jlinT = din("jlinT", [128, N], BF16)
    Ej = din("Ej", [D, D], BF16)
    # packed fp32 params: Ei | Wm | Ai(c-scaled) | Ae(c-scaled) | c*b1 | edge_b
    park = din("park", [D, 4 * D + 2], F32)
    iden = din("iden", [128, 128], BF16)
    lngr = din("lngr", [R, D], F32)
    lnbr = din("lnbr", [R, D], F32)
    out_d = nc.dram_tensor("out", [R, D], F32, kind="ExternalOutput").ap()

    with tile.TileContext(nc) as tc, \
         tc.tile_pool(name="static", bufs=1) as sp:
        # ---------------- static SBUF tiles ----------------
        hT_sb = sp.tile([D, N], F32, name="hT_sb", tag="hT_sb")
        hTr_sb = sp.tile([D, R], F32, name="hTr_sb", tag="hTr_sb")
        hrows_sb = sp.tile([R, D], F32, name="hrows_sb", tag="hrows_sb")
        adjm_sb = sp.tile([R, N], F32, name="adjm_sb", tag="adjm_sb")
        lhsT1_sb = sp.tile([2 * D, DK], BF16, name="lhsT1_sb", tag="lhsT1_sb")
        lhsT2_sb = sp.tile([128, 32 * 128], BF16, name="lhsT2_sb", tag="lhsT2_sb")
        jones_sb = sp.tile([128, DK], BF16, name="jones_sb", tag="jones_sb")
        jlinT_sb = sp.tile([128, N], BF16, name="jlinT_sb", tag="jlinT_sb")
        Ej_sb = sp.tile([D, D], BF16, name="Ej_sb", tag="Ej_sb")
        park_sb = sp.tile([D, 4 * D + 2], F32, name="park_sb", tag="park_sb")
        Ei_sb = park_sb[:, 0:D]
        Wm_sb = park_sb[:, D:2 * D]
        Ai_sb = park_sb[:, 2 * D:3 * D]
        Ae_sb = park_sb[:, 3 * D:4 * D]
        b1_sb = park_sb[:, 4 * D:4 * D + 1]
        eb_sb = park_sb[:, 4 * D + 1:4 * D + 2]
        iden_sb = sp.tile([128, 128], BF16, name="iden_sb", tag="iden_sb")
        lngr_sb = sp.tile([R, D], F32, name="lngr_sb", tag="lngr_sb")
        lnbr_sb = sp.tile([R, D], F32, name="lnbr_sb", tag="lnbr_sb")

        ejT_bf_sb = sp.tile([D, N], BF16, name="ejT_bf_sb", tag="ejT_bf_sb")
        hTbf_sb = sp.tile([D, N], BF16, name="hTbf_sb", tag="hTbf_sb")
        eibr_sb = sp.tile([D, R], F32, name="eibr_sb", tag="eibr_sb")
        WhTr_sb = sp.tile([D, R], F32, name="WhTr_sb", tag="WhTr_sb")
        qb_sb = sp.tile([D, R], F32, name="qb_sb", tag="qb_sb")
        u_sb = sp.tile([DK, R], F32, name="u_sb", tag="u_sb")
        Whb_sb = sp.tile([128, 8 * D], BF16, name="Whb_sb", tag="Whb_sb")
        # rhs1: two i-buffers of [128, N]; rows 64:128 hold hT_bf (constant)
        rhs1_sb = sp.tile([128, 3 * N], BF16, name="rhs1_sb", tag="rhs1_sb")
        rhs2_sb = sp.tile([128, 3 * N], BF16, name="rhs2_sb", tag="rhs2_sb")
        e_sb = sp.tile([R, N], F32, name="e_sb", tag="e_sb")
        em_sb = sp.tile([R, N], F32, name="em_sb", tag="em_sb")
        ex_sb = sp.tile([R, N], F32, name="ex_sb", tag="ex_sb")
        attn_sb = sp.tile([R, N], BF16, name="attn_sb", tag="attn_sb")
        attnT_sb = sp.tile([128, N], BF16, name="attnT_sb", tag="attnT_sb")
        scr_sb = sp.tile([1, 8], F32, name="scr_sb", tag="scr_sb")
        red_sb = sp.tile([R, 8], F32, name="red_sb", tag="red_sb")
        hp_sb = sp.tile([R, D], F32, name="hp_sb", tag="hp_sb")
        xm_sb = sp.tile([R, D], F32, name="xm_sb", tag="xm_sb")
        o_sb = sp.tile([R, D], F32, name="o_sb", tag="o_sb")

        # ---------------- load inputs (critical-path order) ----------------
        # critical DMAs fan out across engine queues for parallel dispatch
        nc.scalar.dma_start(hTbf_sb[:], hT_bf)
        nc.scalar.dma_start(Ej_sb[:], Ej)
        nc.gpsimd.dma_start(hTr_sb[:], hTr)
        nc.gpsimd.dma_start(park_sb[:], park)
        nc.gpsimd.dma_start(lhsT1_sb[:], lhsT1)
        nc.gpsimd.dma_start(jlinT_sb[:], jlinT)
        nc.sync.dma_start(hT_sb[:], hT_f)
        nc.sync.dma_start(lhsT2_sb[:], lhsT2)
        nc.sync.dma_start(jones_sb[:], jones)
        nc.sync.dma_start(rhs1_sb[D:2 * D, 0:N], hT_bf)
        nc.sync.dma_start(rhs1_sb[D:2 * D, N:2 * N], hT_bf)
        nc.sync.dma_start(rhs1_sb[D:2 * D, 2 * N:3 * N], hT_bf)
        nc.sync.dma_start(hrows_sb[:], hrows)
        nc.sync.dma_start(iden_sb[:], iden)
        nc.sync.dma_start(lngr_sb[:], lngr)
        nc.sync.dma_start(lnbr_sb[:], lnbr)
        nc.sync.dma_start(adjm_sb[:], adjm)

        # zero the stage-3 rhs pad rows once (zero weights x junk = NaN risk)
        nc.vector.memset(rhs2_sb[D:128, :], 0.0)

        # warm the ACT exp table (exp/relu/copy share one set)
        nc.vector.memset(scr_sb[:], 1.0)
        nc.scalar.activation(scr_sb[0:1, 0:1], scr_sb[0:1, 1:2], AF.Exp)

        # ---------------- setup math ----------------
        with tc.tile_pool(name="ps_setup", bufs=1, space="PSUM") as psp:
            # ejT (bf16) over all N columns, from the bf16 h copy
            for jh in range(2):
                ej_ps = psp.tile([D, 512], F32, name="ej_ps", bufs=2)
                nc.tensor.matmul(ej_ps[:], Ej_sb[:],
                                 hTbf_sb[:, jh * 512:(jh + 1) * 512])
                nc.vector.tensor_copy(ejT_bf_sb[:, jh * 512:(jh + 1) * 512], ej_ps[:])
            # PE pre-warm: 12 loop-shaped matmuls on loaded data (results unused)
            # so HAM reaches 2.4 GHz before the main loop starts
            wd_ps = psp.tile([DK, 512], F32, name="wd_ps")
            for _ in range(12):
                nc.tensor.matmul(wd_ps[:], lhsT1_sb[:], jlinT_sb[:, 0:512])
            # WhTr = W^T-projected rows (feature-major, this core's columns)
            whtr_ps = psp.tile([D, R], F32, name="whtr_ps", tag="small_ps", bufs=2)
            nc.tensor.matmul(whtr_ps[:], Wm_sb, hTr_sb[:])
            nc.vector.tensor_copy(WhTr_sb[:], whtr_ps[:])
            # eibr = E_i^T h_rows + edge_b
            eib_ps = psp.tile([D, R], F32, name="eib_ps", tag="small_ps", bufs=2)
            nc.tensor.matmul(eib_ps[:], Ei_sb, hTr_sb[:])
            nc.vector.tensor_scalar(eibr_sb[:], eib_ps[:], eb_sb, None, op0=ALU.add)
            # qb = (A_i diag(c))^T WhTr + c*b1
            q_ps = psp.tile([D, R], F32, name="q_ps", tag="small_ps", bufs=2)
            nc.tensor.matmul(q_ps[:], Ai_sb, WhTr_sb[:])
            nc.vector.tensor_scalar(qb_sb[:], q_ps[:], b1_sb, None, op0=ALU.add)
            # u[0:64] = qb + ALPHA * (A_e diag(c))^T eibr ; rows 64,65 = 0
            z_ps = psp.tile([D, R], F32, name="z_ps", tag="small_ps", bufs=2)
            nc.tensor.matmul(z_ps[:], Ae_sb, eibr_sb[:])
            nc.vector.scalar_tensor_tensor(
                u_sb[0:D, :], z_ps[:], ALPHA, qb_sb[:], op0=ALU.mult, op1=ALU.add)
            nc.vector.memset(u_sb[D:DK, :], 0.0)
            # Wh node-major [128, 64] x 8 tiles (bf16 for the final matmul)
            for t in range(8):
                wh_ps = psp.tile([128, D], F32, name="wh_ps", bufs=2)
                nc.tensor.matmul(wh_ps[:], hT_sb[:, t * 128:(t + 1) * 128], Wm_sb)
                nc.vector.tensor_copy(Whb_sb[:, t * D:(t + 1) * D], wh_ps[:])

        # ---------------- main loop over this core's 128 rows ----------------
        def fill(j):
            # rhs1 upper half for row j: relu(ei_j + ejT + eb)
            fb = j % 3
            nc.vector.tensor_scalar(
                rhs1_sb[0:D, fb * N:(fb + 1) * N],
                ejT_bf_sb[:],
                eibr_sb[:, j:j + 1], 0.0, op0=ALU.add, op1=ALU.max)

        with tc.tile_pool(name="ps_mm1", bufs=3, space="PSUM") as pmm1, \
             tc.tile_pool(name="ps_e", bufs=2, space="PSUM") as pe:
            fill(0)
            fill(1)
            bankE = None
            for i in range(R):
                g = i % 32
                grp = i // 32
                buf = i % 3
                if g == 0:
                    bankE = [pe.tile([DK, 512], F32, name="bankE", tag="bankE")
                             for _ in range(2)]
                if i + 2 < R:
                    fill(i + 2)
                # main matmul: psum1[0:64] = c*(t+v) ; rows 64,65 = +-slin
                psum1 = pmm1.tile([DK, N], F32, name="psum1", tag="psum1")
                for jh in range(2):
                    nc.tensor.matmul(
                        psum1[:, jh * 512:(jh + 1) * 512],
                        lhsT1_sb[:],
                        rhs1_sb[:, buf * N + jh * 512: buf * N + (jh + 1) * 512])
                # stage 2: rhs2 = relu(psum1 + u) (bf16), column-split DVE/ACT
                XS = 320
                nc.vector.tensor_scalar(
                    rhs2_sb[0:DK, buf * N:buf * N + XS],
                    psum1[:, 0:XS], u_sb[:, i:i + 1], 0.0,
                    op0=ALU.add, op1=ALU.max)
                nc.scalar.activation(
                    rhs2_sb[0:DK, buf * N + XS:(buf + 1) * N],
                    psum1[:, XS:N], AF.Relu,
                    bias=u_sb[:, i:i + 1], scale=1.0)
                # score matmul: accumulate e rows into persistent banks
                for jh in range(2):
                    nc.tensor.matmul(
                        bankE[jh][:],
                        lhsT2_sb[:, g * 128:g * 128 + DK],
                        rhs2_sb[:, buf * N + jh * 512: buf * N + (jh + 1) * 512],
                        start=(g == 0), stop=False)
                if g == 31:
                    # add per-j linear part (jlin) to all 32 rows, close group
                    for jh in range(2):
                        nc.tensor.matmul(
                            bankE[jh][:],
                            jones_sb[:],
                            jlinT_sb[:, jh * 512:(jh + 1) * 512],
                            start=False, stop=True)
                    for jh in range(2):
                        dst = e_sb[grp * 32:(grp + 1) * 32,
                                   jh * 512:(jh + 1) * 512]
                        if (grp + jh) % 2 == 0:
                            nc.vector.tensor_copy(dst, bankE[jh][0:32, :])
                        else:
                            nc.scalar.copy(dst, bankE[jh][0:32, :])

        # ---------------- mask + softmax ----------------
        nc.vector.tensor_tensor(em_sb[:], e_sb[:], adjm_sb[:], op=ALU.add)
        nc.vector.reduce_max(red_sb[:, 0:1], em_sb[:], axis=AX.X)
        nc.vector.tensor_scalar(red_sb[:, 1:2], red_sb[:, 0:1], -1.0, None,
                                op0=ALU.mult)
        nc.scalar.activation(ex_sb[:], em_sb[:], AF.Exp,
                             bias=red_sb[:, 1:2], scale=1.0,
                             accum_out=red_sb[:, 2:3])
        nc.vector.reciprocal(red_sb[:, 3:4], red_sb[:, 2:3])
        nc.vector.tensor_scalar(attn_sb[:], ex_sb[:], red_sb[:, 3:4], None,
                                op0=ALU.mult)

        # ---------------- h' = attn @ Wh + h ; LayerNorm ----------------
        with tc.tile_pool(name="ps_fin", bufs=4, space="PSUM") as pf:
            for t in range(8):
                tp_ps = pf.tile([128, 128], BF16, name="tp_ps", tag="tp")
                nc.tensor.transpose(tp_ps[:], attn_sb[:, t * 128:(t + 1) * 128],
                                    iden_sb[:])
                nc.vector.tensor_copy(attnT_sb[:, t * 128:(t + 1) * 128], tp_ps[:])
            hp_ps = pf.tile([R, D], F32, name="hp_ps", bufs=1)
            for t in range(8):
                nc.tensor.matmul(hp_ps[:], attnT_sb[:, t * 128:(t + 1) * 128],
                                 Whb_sb[:, t * D:(t + 1) * D],
                                 start=(t == 0), stop=(t == 7))
            nc.vector.tensor_tensor(hp_sb[:], hp_ps[:], hrows_sb[:], op=ALU.add)

        nc.vector.reduce_sum(red_sb[:, 4:5], hp_sb[:], axis=AX.X)
        nc.vector.tensor_scalar(red_sb[:, 5:6], red_sb[:, 4:5], 1.0 / D, None,
                                op0=ALU.mult)
        nc.vector.tensor_scalar(xm_sb[:], hp_sb[:], red_sb[:, 5:6], None,
                                op0=ALU.subtract)
        nc.vector.tensor_tensor(o_sb[:], xm_sb[:], xm_sb[:], op=ALU.mult)
        nc.vector.reduce_sum(red_sb[:, 6:7], o_sb[:], axis=AX.X)
        # rstd = 1 / sqrt(var + eps)
        nc.vector.tensor_scalar(red_sb[:, 6:7], red_sb[:, 6:7], 1.0 / D,
                                LN_EPS, op0=ALU.mult, op1=ALU.add)
        nc.scalar.activation(red_sb[:, 7:8], red_sb[:, 6:7], AF.Sqrt)
        nc.vector.reciprocal(red_sb[:, 7:8], red_sb[:, 7:8])
        nc.vector.tensor_scalar(xm_sb[:], xm_sb[:], red_sb[:, 7:8], None,
                                op0=ALU.mult)
        nc.vector.tensor_tensor(o_sb[:], xm_sb[:], lngr_sb[:], op=ALU.mult)
        nc.vector.tensor_tensor(o_sb[:], o_sb[:], lnbr_sb[:], op=ALU.add)
        nc.sync.dma_start(out_d, o_sb[:])

    nc.compile()
    return nc


def _host_prep(inputs):
    h = np.asarray(inputs["h"], np.float32)[0]            # [N, D]
    adj = np.asarray(inputs["adj"])[0]                    # [N, N] int32
    W = np.asarray(inputs["W"], np.float32)
    attn_w1 = np.asarray(inputs["attn_w1"], np.float32)
    attn_b1 = np.asarray(inputs["attn_b1"], np.float32)
    attn_w2 = np.asarray(inputs["attn_w2"], np.float32)
    edge_w = np.asarray(inputs["edge_w"], np.float32)
    edge_b = np.asarray(inputs["edge_b"], np.float32)
    ln_g = np.asarray(inputs["ln_g"], np.float32)
    ln_b = np.asarray(inputs["ln_b"], np.float32)

    A_i, A_j, A_e = attn_w1[:D], attn_w1[D:2 * D], attn_w1[2 * D:]
    E_i, E_j = edge_w[:D], edge_w[D:]
    w2 = attn_w2[:, 0]

    hT = np.ascontiguousarray(h.T)                        # [D, N]
    Mv = W @ A_j + ALPHA * (E_j @ A_e)
    c = 0.8 * np.abs(w2)
    sgn = np.sign(w2).astype(np.float32)

    lhsT1 = np.zeros((2 * D, DK), np.float32)
    lhsT1[:D, :D] = 0.8 * A_e * c[None, :]
    lhsT1[D:, :D] = Mv * c[None, :]
    slw = 0.8 * ALPHA * (A_e @ w2)
    lhsT1[:D, D] = slw
    lhsT1[:D, D + 1] = -slw

    lhsT2 = np.zeros((128, 32 * 128), np.float32)
    for g in range(32):
        lhsT2[:D, g * 128 + g] = sgn
        lhsT2[D, g * 128 + g] = 1.0
        lhsT2[D + 1, g * 128 + g] = -1.0

    jones = np.zeros((128, DK), np.float32)
    jones[0, 0:32] = 1.0

    jlin = ALPHA * (h @ (Mv @ w2))                        # [N]
    jlinT = np.zeros((128, N), np.float32)
    jlinT[0] = jlin

    rep = {
        "hT_f": hT,
        "hT_bf": hT.astype(ml_dtypes.bfloat16),
        "lhsT1": lhsT1.astype(ml_dtypes.bfloat16),
        "lhsT2": lhsT2.astype(ml_dtypes.bfloat16),
        "jones": jones.astype(ml_dtypes.bfloat16),
        "jlinT": jlinT.astype(ml_dtypes.bfloat16),
        "Ej": np.ascontiguousarray(E_j).astype(ml_dtypes.bfloat16),
        "park": np.concatenate(
            [E_i, W, A_i * c[None, :], A_e * c[None, :],
             (c * attn_b1)[:, None], edge_b[:, None]], axis=1).astype(np.float32),
        "iden": np.eye(128, dtype=ml_dtypes.bfloat16),
        "lngr": np.broadcast_to(ln_g, (R, D)).copy(),
        "lnbr": np.broadcast_to(ln_b, (R, D)).copy(),
    }
    in_maps = []
    for cid in range(NCORES):
        rows = slice(cid * R, (cid + 1) * R)
        m = dict(rep)
        m["hTr"] = np.ascontiguousarray(hT[:, rows])
        m["hrows"] = np.ascontiguousarray(h[rows])
        m["adjm"] = np.where(adj[rows] == 0, np.float32(-30000.0),
                             np.float32(0.0))
        in_maps.append(m)
    return in_maps


def _get_nc():
    if "nc" not in _CACHE:
        _CACHE["nc"] = _build_program()
    return _CACHE["nc"]


def kernel(**inputs) -> np.ndarray:
    nc = _get_nc()
    in_maps = _host_prep(inputs)
    res = run_bass_kernel_spmd(nc, in_maps, list(range(NCORES))).results
    out = np.concatenate([res[c]["out"] for c in range(NCORES)], axis=0)
    return out[None].astype(np.float32)


# revision 18
# speedup vs baseline: 4.2307x; 1.0403x over previous
"""Trainium2 Bass kernel for EnhancedGraphAttentionLayer (B=1, N=1024, D=64).

Sharding: destination-node rows split across 8 cores (128 rows each).
Each core is fully independent (no collectives): it holds h replicated and
computes its 128 rows of scores/softmax/attention locally.

v2 decomposition (all-bf16 matmuls):
  LeakyReLU(x) = 0.2*x + 0.8*relu(x) at both nonlinearities.
  pre_ij = u_i + 0.8*A_e^T relu(s_ij) + Mv^T h_j,  s_ij = ei_i + ej_j + eb
  e_ij  = 0.8*sum_k w2_k relu(pre_k) + 0.2*w2^T(t+v) + const_i
  Fold c_k = 0.8*|w2_k| into stage-1 weight cols + u rows (LeakyReLU is
  positively homogeneous), so stage-3 reduces with exact +-1 signs in bf16.
  The 0.2-linear score part splits into:
    slin (from relu(s)): carried as two extra +-slin psum rows
      (relu(x)-relu(-x)=x, exact), reduced with +1/-1 in stage-3;
    jlin (per-j const): host-precomputed row vector, added into the score
      psum bank with one ones-weight matmul pair per 32-row group.
  Mask is additive (-30000 on adj==0), so no relu-safe score shift needed.
"""
import sys
import numpy as np

if "/opt/trn_rl_repo" not in sys.path:
    sys.path.insert(0, "/opt/trn_rl_repo")

import ml_dtypes
import concourse.bass as bass
import concourse.bacc as bacc
import concourse.mybir as mybir
import concourse.tile as tile
from concourse.bass_utils import run_bass_kernel_spmd

F32 = mybir.dt.float32
BF16 = mybir.dt.bfloat16
AF = mybir.ActivationFunctionType
ALU = mybir.AluOpType
AX = mybir.AxisListType

N = 1024
D = 64
NCORES = 8
R = N // NCORES          # 128 rows per core
ALPHA = 0.2
LN_EPS = 1e-5
DK = D + 2               # 64 features + slin+/- rows

_CACHE = {}


def _build_program():
    nc = bacc.Bacc("TRN2", target_bir_lowering=False, debug=False,
                   num_devices=NCORES)

    def din(name, shape, dt):
        return nc.dram_tensor(name, shape, dt, kind="ExternalInput").ap()

    hT_f = din("hT_f", [D, N], F32)
    hT_bf = din("hT_bf", [D, N], BF16)
    hTr = din("hTr", [D, R], F32)
    hrows = din("hrows", [R, D], F32)
    adjm = din("adjm", [R, N], F32)
    lhsT1 = din("lhsT1", [2 * D, DK], BF16)
    # stage-3 weights padded to the same [128, 66] shape as lhsT1 so the PE
    # never switches weight tile shape (shape alternation blocks HAM warm-up)
    lhsT2 = din("lhsT2", [128, 32 * 128], BF16)
    jones = din("jones", [128, DK], BF16)
    jlinT = din("jlinT", [128, N], BF16)
    Ej = din("Ej", [D, D], BF16)
    # packed fp32 params: Ei | Wm | Ai(c-scaled) | Ae(c-scaled) | c*b1 | edge_b
    park = din("park", [D, 4 * D + 2], F32)
    iden = din("iden", [128, 128], BF16)
    lngr = din("lngr", [R, D], F32)
    lnbr = din("lnbr", [R, D], F32)
    out_d = nc.dram_tensor("out", [R, D], F32, kind="ExternalOutput").ap()

    with tile.TileContext(nc) as tc, \
         tc.tile_pool(name="static", bufs=1) as sp:
        # ---------------- static SBUF tiles ----------------
        hT_sb = sp.tile([D, N], F32, name="hT_sb", tag="hT_sb")
        hTr_sb = sp.tile([D, R], F32, name="hTr_sb", tag="hTr_sb")
        hrows_sb = sp.tile([R, D], F32, name="hrows_sb", tag="hrows_sb")
        adjm_sb = sp.tile([R, N], F32, name="adjm_sb", tag="adjm_sb")
        lhsT1_sb = sp.tile([2 * D, DK], BF16, name="lhsT1_sb", tag="lhsT1_sb")
        lhsT2_sb = sp.tile([128, 32 * 128], BF16, name="lhsT2_sb", tag="lhsT2_sb")
        jones_sb = sp.tile([128, DK], BF16, name="jones_sb", tag="jones_sb")
        jlinT_sb = sp.tile([128, N], BF16, name="jlinT_sb", tag="jlinT_sb")
        Ej_sb = sp.tile([D, D], BF16, name="Ej_sb", tag="Ej_sb")
        park_sb = sp.tile([D, 4 * D + 2], F32, name="park_sb", tag="park_sb")
        Ei_sb = park_sb[:, 0:D]
        Wm_sb = park_sb[:, D:2 * D]
        Ai_sb = park_sb[:, 2 * D:3 * D]
        Ae_sb = park_sb[:, 3 * D:4 * D]
        b1_sb = park_sb[:, 4 * D:4 * D + 1]
        eb_sb = park_sb[:, 4 * D + 1:4 * D + 2]
        iden_sb = sp.tile([128, 128], BF16, name="iden_sb", tag="iden_sb")
        lngr_sb = sp.tile([R, D], F32, name="lngr_sb", tag="lngr_sb")
        lnbr_sb = sp.tile([R, D], F32, name="lnbr_sb", tag="lnbr_sb")

        ejT_bf_sb = sp.tile([D, N], BF16, name="ejT_bf_sb", tag="ejT_bf_sb")
        hTbf_sb = sp.tile([D, N], BF16, name="hTbf_sb", tag="hTbf_sb")
        eibr_sb = sp.tile([D, R], F32, name="eibr_sb", tag="eibr_sb")
        WhTr_sb = sp.tile([D, R], F32, name="WhTr_sb", tag="WhTr_sb")
        qb_sb = sp.tile([D, R], F32, name="qb_sb", tag="qb_sb")
        u_sb = sp.tile([DK, R], F32, name="u_sb", tag="u_sb")
        Whb_sb = sp.tile([128, 8 * D], BF16, name="Whb_sb", tag="Whb_sb")
        # rhs1: two i-buffers of [128, N]; rows 64:128 hold hT_bf (constant)
        rhs1_sb = sp.tile([128, 3 * N], BF16, name="rhs1_sb", tag="rhs1_sb")
        rhs2_sb = sp.tile([128, 3 * N], BF16, name="rhs2_sb", tag="rhs2_sb")
        e_sb = sp.tile([R, N], F32, name="e_sb", tag="e_sb")
        em_sb = sp.tile([R, N], F32, name="em_sb", tag="em_sb")
        ex_sb = sp.tile([R, N], F32, name="ex_sb", tag="ex_sb")
        attn_sb = sp.tile([R, N], BF16, name="attn_sb", tag="attn_sb")
        attnT_sb = sp.tile([128, N], BF16, name="attnT_sb", tag="attnT_sb")
        scr_sb = sp.tile([1, 8], F32, name="scr_sb", tag="scr_sb")
        red_sb = sp.tile([R, 8], F32, name="red_sb", tag="red_sb")
        hp_sb = sp.tile([R, D], F32, name="hp_sb", tag="hp_sb")
        xm_sb = sp.tile([R, D], F32, name="xm_sb", tag="xm_sb")
        o_sb = sp.tile([R, D], F32, name="o_sb", tag="o_sb")

        # ---------------- load inputs (critical-path order) ----------------
        nc.sync.dma_start(hTbf_sb[:], hT_bf)
        nc.sync.dma_start(Ej_sb[:], Ej)
        nc.sync.dma_start(hTr_sb[:], hTr)
        nc.sync.dma_start(park_sb[:], park)
        nc.sync.dma_start(lhsT1_sb[:], lhsT1)
        nc.sync.dma_start(jlinT_sb[:], jlinT)
        nc.sync.dma_start(hT_sb[:], hT_f)
        nc.sync.dma_start(lhsT2_sb[:], lhsT2)
        nc.sync.dma_start(jones_sb[:], jones)
        nc.sync.dma_start(rhs1_sb[D:2 * D, 0:N], hT_bf)
        nc.sync.dma_start(rhs1_sb[D:2 * D, N:2 * N], hT_bf)
        nc.sync.dma_start(rhs1_sb[D:2 * D, 2 * N:3 * N], hT_bf)
        nc.sync.dma_start(hrows_sb[:], hrows)
        nc.sync.dma_start(iden_sb[:], iden)
        nc.sync.dma_start(lngr_sb[:], lngr)
        nc.sync.dma_start(lnbr_sb[:], lnbr)
        nc.sync.dma_start(adjm_sb[:], adjm)

        # zero the stage-3 rhs pad rows once (zero weights x junk = NaN risk)
        nc.vector.memset(rhs2_sb[D:128, :], 0.0)

        # warm the ACT exp table (exp/relu/copy share one set)
        nc.vector.memset(scr_sb[:], 1.0)
        nc.scalar.activation(scr_sb[0:1, 0:1], scr_sb[0:1, 1:2], AF.Exp)

        # ---------------- setup math ----------------
        with tc.tile_pool(name="ps_setup", bufs=1, space="PSUM") as psp:
            # ejT (bf16) over all N columns, from the bf16 h copy
            for jh in range(2):
                ej_ps = psp.tile([D, 512], F32, name="ej_ps", bufs=2)
                nc.tensor.matmul(ej_ps[:], Ej_sb[:],
                                 hTbf_sb[:, jh * 512:(jh + 1) * 512])
                nc.vector.tensor_copy(ejT_bf_sb[:, jh * 512:(jh + 1) * 512], ej_ps[:])
            # PE pre-warm: 12 loop-shaped matmuls on loaded data (results unused)
            # so HAM reaches 2.4 GHz before the main loop starts
            wd_ps = psp.tile([DK, 512], F32, name="wd_ps")
            for _ in range(12):
                nc.tensor.matmul(wd_ps[:], lhsT1_sb[:], jlinT_sb[:, 0:512])
            # WhTr = W^T-projected rows (feature-major, this core's columns)
            whtr_ps = psp.tile([D, R], F32, name="whtr_ps", tag="small_ps", bufs=2)
            nc.tensor.matmul(whtr_ps[:], Wm_sb, hTr_sb[:])
            nc.vector.tensor_copy(WhTr_sb[:], whtr_ps[:])
            # eibr = E_i^T h_rows + edge_b
            eib_ps = psp.tile([D, R], F32, name="eib_ps", tag="small_ps", bufs=2)
            nc.tensor.matmul(eib_ps[:], Ei_sb, hTr_sb[:])
            nc.vector.tensor_scalar(eibr_sb[:], eib_ps[:], eb_sb, None, op0=ALU.add)
            # qb = (A_i diag(c))^T WhTr + c*b1
            q_ps = psp.tile([D, R], F32, name="q_ps", tag="small_ps", bufs=2)
            nc.tensor.matmul(q_ps[:], Ai_sb, WhTr_sb[:])
            nc.vector.tensor_scalar(qb_sb[:], q_ps[:], b1_sb, None, op0=ALU.add)
            # u[0:64] = qb + ALPHA * (A_e diag(c))^T eibr ; rows 64,65 = 0
            z_ps = psp.tile([D, R], F32, name="z_ps", tag="small_ps", bufs=2)
            nc.tensor.matmul(z_ps[:], Ae_sb, eibr_sb[:])
            nc.vector.scalar_tensor_tensor(
                u_sb[0:D, :], z_ps[:], ALPHA, qb_sb[:], op0=ALU.mult, op1=ALU.add)
            nc.vector.memset(u_sb[D:DK, :], 0.0)
            # Wh node-major [128, 64] x 8 tiles (bf16 for the final matmul)
            for t in range(8):
                wh_ps = psp.tile([128, D], F32, name="wh_ps", bufs=2)
                nc.tensor.matmul(wh_ps[:], hT_sb[:, t * 128:(t + 1) * 128], Wm_sb)
                nc.vector.tensor_copy(Whb_sb[:, t * D:(t + 1) * D], wh_ps[:])

        # ---------------- main loop over this core's 128 rows ----------------
        def fill(j):
            # rhs1 upper half for row j: relu(ei_j + ejT + eb)
            fb = j % 3
            nc.vector.tensor_scalar(
                rhs1_sb[0:D, fb * N:(fb + 1) * N],
                ejT_bf_sb[:],
                eibr_sb[:, j:j + 1], 0.0, op0=ALU.add, op1=ALU.max)

        with tc.tile_pool(name="ps_mm1", bufs=3, space="PSUM") as pmm1, \
             tc.tile_pool(name="ps_e", bufs=2, space="PSUM") as pe:
            fill(0)
            fill(1)
            bankE = None
            for i in range(R):
                g = i % 32
                grp = i // 32
                buf = i % 3
                if g == 0:
                    bankE = [pe.tile([DK, 512], F32, name="bankE", tag="bankE")
                             for _ in range(2)]
                if i + 2 < R:
                    fill(i + 2)
                # main matmul: psum1[0:64] = c*(t+v) ; rows 64,65 = +-slin
                psum1 = pmm1.tile([DK, N], F32, name="psum1", tag="psum1")
                for jh in range(2):
                    nc.tensor.matmul(
                        psum1[:, jh * 512:(jh + 1) * 512],
                        lhsT1_sb[:],
                        rhs1_sb[:, buf * N + jh * 512: buf * N + (jh + 1) * 512])
                # stage 2: rhs2 = relu(psum1 + u) (bf16), column-split DVE/ACT
                XS = 320
                nc.vector.tensor_scalar(
                    rhs2_sb[0:DK, buf * N:buf * N + XS],
                    psum1[:, 0:XS], u_sb[:, i:i + 1], 0.0,
                    op0=ALU.add, op1=ALU.max)
                nc.scalar.activation(
                    rhs2_sb[0:DK, buf * N + XS:(buf + 1) * N],
                    psum1[:, XS:N], AF.Relu,
                    bias=u_sb[:, i:i + 1], scale=1.0)
                # score matmul: accumulate e rows into persistent banks
                for jh in range(2):
                    nc.tensor.matmul(
                        bankE[jh][:],
                        lhsT2_sb[:, g * 128:g * 128 + DK],
                        rhs2_sb[:, buf * N + jh * 512: buf * N + (jh + 1) * 512],
                        start=(g == 0), stop=False)
                if g == 31:
                    # add per-j linear part (jlin) to all 32 rows, close group
                    for jh in range(2):
                        nc.tensor.matmul(
                            bankE[jh][:],
                            jones_sb[:],
                            jlinT_sb[:, jh * 512:(jh + 1) * 512],
                            start=False, stop=True)
                    for jh in range(2):
                        dst = e_sb[grp * 32:(grp + 1) * 32,
                                   jh * 512:(jh + 1) * 512]
                        if (grp + jh) % 2 == 0:
                            nc.vector.tensor_copy(dst, bankE[jh][0:32, :])
                        else:
                            nc.scalar.copy(dst, bankE[jh][0:32, :])

        # ---------------- mask + softmax ----------------
        nc.vector.tensor_tensor(em_sb[:], e_sb[:], adjm_sb[:], op=ALU.add)
        nc.vector.reduce_max(red_sb[:, 0:1], em_sb[:], axis=AX.X)
        nc.vector.tensor_scalar(red_sb[:, 1:2], red_sb[:, 0:1], -1.0, None,
                                op0=ALU.mult)
        nc.scalar.activation(ex_sb[:], em_sb[:], AF.Exp,
                             bias=red_sb[:, 1:2], scale=1.0,
                             accum_out=red_sb[:, 2:3])
        nc.vector.reciprocal(red_sb[:, 3:4], red_sb[:, 2:3])
        nc.vector.tensor_scalar(attn_sb[:], ex_sb[:], red_sb[:, 3:4], None,
                                op0=ALU.mult)

        # ---------------- h' = attn @ Wh + h ; LayerNorm ----------------
        with tc.tile_pool(name="ps_fin", bufs=4, space="PSUM") as pf:
            for t in range(8):
                tp_ps = pf.tile([128, 128], BF16, name="tp_ps", tag="tp")
                nc.tensor.transpose(tp_ps[:], attn_sb[:, t * 128:(t + 1) * 128],
                                    iden_sb[:])
                nc.vector.tensor_copy(attnT_sb[:, t * 128:(t + 1) * 128], tp_ps[:])
            hp_ps = pf.tile([R, D], F32, name="hp_ps", bufs=1)
            for t in range(8):
                nc.tensor.matmul(hp_ps[:], attnT_sb[:, t * 128:(t + 1) * 128],
                                 Whb_sb[:, t * D:(t + 1) * D],
                                 start=(t == 0), stop=(t == 7))
            nc.vector.tensor_tensor(hp_sb[:], hp_ps[:], hrows_sb[:], op=ALU.add)

        nc.vector.reduce_sum(red_sb[:, 4:5], hp_sb[:], axis=AX.X)
        nc.vector.tensor_scalar(red_sb[:, 5:6], red_sb[:, 4:5], 1.0 / D, None,
                                op0=ALU.mult)
        nc.vector.tensor_scalar(xm_sb[:], hp_sb[:], red_sb[:, 5:6], None,
                                op0=ALU.subtract)
        nc.vector.tensor_tensor(o_sb[:], xm_sb[:], xm_sb[:], op=ALU.mult)
        nc.vector.reduce_sum(red_sb[:, 6:7], o_sb[:], axis=AX.X)
        # rstd = 1 / sqrt(var + eps)
        nc.vector.tensor_scalar(red_sb[:, 6:7], red_sb[:, 6:7], 1.0 / D,
                                LN_EPS, op0=ALU.mult, op1=ALU.add)
        nc.scalar.activation(red_sb[:, 7:8], red_sb[:, 6:7], AF.Sqrt)
        nc.vector.reciprocal(red_sb[:, 7:8], red_sb[:, 7:8])
        nc.vector.tensor_scalar(xm_sb[:], xm_sb[:], red_sb[:, 7:8], None,
                                op0=ALU.mult)
        nc.vector.tensor_tensor(o_sb[:], xm_sb[:], lngr_sb[:], op=ALU.mult)
        nc.vector.tensor_tensor(o_sb[:], o_sb[:], lnbr_sb[:], op=ALU.add)
        nc.sync.dma_start(out_d, o_sb[:])

    nc.compile()
    return nc


def _host_prep(inputs):
    h = np.asarray(inputs["h"], np.float32)[0]            # [N, D]
    adj = np.asarray(inputs["adj"])[0]                    # [N, N] int32
    W = np.asarray(inputs["W"], np.float32)
    attn_w1 = np.asarray(inputs["attn_w1"], np.float32)
    attn_b1 = np.asarray(inputs["attn_b1"], np.float32)
    attn_w2 = np.asarray(inputs["attn_w2"], np.float32)
    edge_w = np.asarray(inputs["edge_w"], np.float32)
    edge_b = np.asarray(inputs["edge_b"], np.float32)
    ln_g = np.asarray(inputs["ln_g"], np.float32)
    ln_b = np.asarray(inputs["ln_b"], np.float32)

    A_i, A_j, A_e = attn_w1[:D], attn_w1[D:2 * D], attn_w1[2 * D:]
    E_i, E_j = edge_w[:D], edge_w[D:]
    w2 = attn_w2[:, 0]

    hT = np.ascontiguousarray(h.T)                        # [D, N]
    Mv = W @ A_j + ALPHA * (E_j @ A_e)
    c = 0.8 * np.abs(w2)
    sgn = np.sign(w2).astype(np.float32)

    lhsT1 = np.zeros((2 * D, DK), np.float32)
    lhsT1[:D, :D] = 0.8 * A_e * c[None, :]
    lhsT1[D:, :D] = Mv * c[None, :]
    slw = 0.8 * ALPHA * (A_e @ w2)
    lhsT1[:D, D] = slw
    lhsT1[:D, D + 1] = -slw

    lhsT2 = np.zeros((128, 32 * 128), np.float32)
    for g in range(32):
        lhsT2[:D, g * 128 + g] = sgn
        lhsT2[D, g * 128 + g] = 1.0
        lhsT2[D + 1, g * 128 + g] = -1.0

    jones = np.zeros((128, DK), np.float32)
    jones[0, 0:32] = 1.0

    jlin = ALPHA * (h @ (Mv @ w2))                        # [N]
    jlinT = np.zeros((128, N), np.float32)
    jlinT[0] = jlin

    rep = {
        "hT_f": hT,
        "hT_bf": hT.astype(ml_dtypes.bfloat16),
        "lhsT1": lhsT1.astype(ml_dtypes.bfloat16),
        "lhsT2": lhsT2.astype(ml_dtypes.bfloat16),
        "jones": jones.astype(ml_dtypes.bfloat16),
        "jlinT": jlinT.astype(ml_dtypes.bfloat16),
        "Ej": np.ascontiguousarray(E_j).astype(ml_dtypes.bfloat16),
        "park": np.concatenate(
            [E_i, W, A_i * c[None, :], A_e * c[None, :],
             (c * attn_b1)[:, None], edge_b[:, None]], axis=1).astype(np.float32),
        "iden": np.eye(128, dtype=ml_dtypes.bfloat16),
        "lngr": np.broadcast_to(ln_g, (R, D)).copy(),
        "lnbr": np.broadcast_to(ln_b, (R, D)).copy(),
    }
    in_maps = []
    for cid in range(NCORES):
        rows = slice(cid * R, (cid + 1) * R)
        m = dict(rep)
        m["hTr"] = np.ascontiguousarray(hT[:, rows])
        m["hrows"] = np.ascontiguousarray(h[rows])
        m["adjm"] = np.where(adj[rows] == 0, np.float32(-30000.0),
                             np.float32(0.0))
        in_maps.append(m)
    return in_maps


def _get_nc():
    if "nc" not in _CACHE:
        _CACHE["nc"] = _build_program()
    return _CACHE["nc"]


def kernel(**inputs) -> np.ndarray:
    nc = _get_nc()
    in_maps = _host_prep(inputs)
    res = run_bass_kernel_spmd(nc, in_maps, list(range(NCORES))).results
    out = np.concatenate([res[c]["out"] for c in range(NCORES)], axis=0)
    return out[None].astype(np.float32)
